# revision 18
# baseline (speedup 1.0000x reference)
"""Trainium2 Bass kernel for GQA attention (B=2, T=2048, C=4096, H=32, KV=8, D=128)
with RoPE and causal mask.

Sharding: tensor-parallel over heads across 8 cores. Each core owns 4 Q heads and
their shared KV head: projects q/k/v for those heads, runs causal attention, and
computes a partial output projection; the host sums the 8 partials (bf16).

All on-chip layouts are transposed ([feature, token]) so every matmul consumes
natural slices:
  qT/kT/vT = W^T @ x  via lhsT=W-tile [128c, cols], rhs=xT-tile [128c, 512t]
  sT[tk, tq] = kT-tile^T @ qT-chunk, two k-tiles paired into one [128,1024]
  PSUM tile so ACT computes exp on both halves in one ACTIVATE
  pT = exp(sT/sqrt(D) - 10); strictly-causal-upper tiles skipped entirely
  yT[d, tq] += v-tile^T @ pT          (v pre-transposed to [t, d] via PE transpose)
  out[tq, :] += yT_h^T @ wo_h         (accumulate 4 heads in PSUM, evict bf16)

Softmax denominator: the exp tiles are pre-summed across key-tiles on DVE
(heads 0,2) / GPSIMD (heads 1,3) into a bf16 row-sum; a single ones-matmul per
(head, chunk) reduces it across partitions. This removes the per-key-tile
ones-matmul stream (~9% of PE cycles) from the tensor engine.

PSUM is managed as ONE kernel-wide pool of four 2-bank [128,1024] tags so no
pool-release barrier ever serializes phase transitions (per-address WAR deps
only): banks 0-1 = pq0/pq1 then yps/dps; banks 2-3 = pq2/pq3 then the wo-job
accumulators; banks 4-5 = pk/pv then odd score pairs; banks 6-7 = V-transpose
ping-pong then even score pairs. At the end of each batch's projections the
V transposes + their evictions are emitted BEFORE the last chunk's RoPE
evictions, so the first attention scores (banks 6-7) start within ~4us of the
last projection matmul and the PE never goes HAM-cold at the phase boundary.
Output-projection matmul "jobs" are popped from a queue inside the score
streams to keep the in-order PE queue dense while ACT works through the exps.
"""

import os
from collections import deque
from contextlib import ExitStack

import numpy as np
import ml_dtypes

import concourse.bacc as bacc
import concourse.mybir as mybir
import concourse.tile as tile

BF = mybir.dt.bfloat16
F32 = mybir.dt.float32
AFT = mybir.ActivationFunctionType

NCORES = 8
B, T, C = 2, 2048, 4096
H, KV, D = 32, 8, 128
QH = H // NCORES          # 4 q-heads per core
CT = C // 128             # 32 contraction tiles
NCH = T // 512            # 4 query chunks per batch
SCALE = 1.0 / float(np.sqrt(D))
EXP_BIAS = -10.0
ROPE_BASE = 10000.0

bf16 = ml_dtypes.bfloat16


def emit_program():
    nc = bacc.Bacc("TRN2", target_bir_lowering=False, debug=False,
                   num_devices=NCORES)

    xT_d = nc.dram_tensor("xT", [C, B * T], BF, kind="ExternalInput").ap()
    wq_d = nc.dram_tensor("wq", [C, QH * D], BF, kind="ExternalInput").ap()
    wk_d = nc.dram_tensor("wk", [C, D], BF, kind="ExternalInput").ap()
    wv_d = nc.dram_tensor("wv", [C, D], BF, kind="ExternalInput").ap()
    wo_d = nc.dram_tensor("woA", [128, QH, C], BF, kind="ExternalInput").ap()
    cos_d = nc.dram_tensor("cosT", [D, T], BF, kind="ExternalInput").ap()
    sin_d = nc.dram_tensor("sinTr", [D, T], BF, kind="ExternalInput").ap()
    alw_d = nc.dram_tensor("allowP", [128, 2, 1024], BF, kind="ExternalInput").ap()
    id_d = nc.dram_tensor("ident", [128, 128], BF, kind="ExternalInput").ap()
    out_d = nc.dram_tensor("out", [B * T, C], BF, kind="ExternalOutput").ap()

    with tile.TileContext(nc) as tc, ExitStack() as ctx:
        const = ctx.enter_context(tc.tile_pool(name="const", bufs=1))
        act = ctx.enter_context(tc.tile_pool(name="act", bufs=1))
        work = ctx.enter_context(tc.tile_pool(name="work", bufs=1))
        ps = ctx.enter_context(tc.tile_pool(name="ps", bufs=1, space="PSUM"))

        def bank2(tag):
            return ps.tile([128, 1024], F32, tag=tag, bufs=1, name=tag)

        # weights + tables on the gpsimd DMA queue so they never sit ahead of
        # the xt activation loads (sync queue); chunked in 8-c-tile groups so
        # the first projection matmuls wait on ~1MB, not the full tensors
        wq_sb = const.tile([128, CT, QH * D], BF)
        wk_sb = const.tile([128, CT, D], BF)
        wv_sb = const.tile([128, CT, D], BF)
        wqr = wq_d.rearrange("(ci p) n -> p ci n", p=128)
        wkr = wk_d.rearrange("(ci p) n -> p ci n", p=128)
        wvr = wv_d.rearrange("(ci p) n -> p ci n", p=128)
        GW = 8
        for g in range(0, CT, GW):
            s = slice(g, g + GW)
            nc.gpsimd.dma_start(wq_sb[:, s, :], wqr[:, s, :])
            nc.gpsimd.dma_start(wk_sb[:, s, :], wkr[:, s, :])
            nc.gpsimd.dma_start(wv_sb[:, s, :], wvr[:, s, :])
        cos_sb = const.tile([D, T], BF)
        nc.gpsimd.dma_start(cos_sb[:], cos_d)
        sin_sb = const.tile([D, T], BF)
        nc.gpsimd.dma_start(sin_sb[:], sin_d)
        alw_sb = const.tile([128, 2, 1024], BF)
        nc.gpsimd.dma_start(alw_sb[:], alw_d)
        id_sb = const.tile([128, 128], BF)
        nc.gpsimd.dma_start(id_sb[:], id_d)
        wo_sb = const.tile([128, QH, C], BF)
        nc.gpsimd.dma_start(wo_sb[:], wo_d)
        onesbf_sb = const.tile([128, 128], BF)
        nc.gpsimd.memset(onesbf_sb[:], 1.0)
        bias_sb = const.tile([128, 1], F32)
        nc.gpsimd.memset(bias_sb[:], EXP_BIAS)

        def rope_evict(dst, psum, cs):
            # dst = psum * cos + swap_halves(psum) * sin_rot   (bf16 out)
            # half-swap copies on ACT (fast PSUM reads), muls/add on DVE
            sw = work.tile([128, 512], F32, tag="sw", bufs=3, name="sw")
            nc.scalar.copy(sw[0:64, :], psum[64:128, :])
            nc.scalar.copy(sw[64:128, :], psum[0:64, :])
            nc.vector.tensor_mul(sw[:], sw[:], sin_sb[:, cs])
            cst = work.tile([128, 512], F32, tag="cst", bufs=3, name="cst")
            nc.vector.tensor_mul(cst[:], psum[:], cos_sb[:, cs])
            nc.vector.tensor_add(dst, cst[:], sw[:])

        wo_jobs = deque()
        # wo-job PSUM accumulators ping-pong over half-bank slots; during the
        # end-of-batch drain the idle score banks join the rotation so PE
        # never waits for an eviction
        ops_holder = {"tags": ["bk23"], "i": 0, "tiles": {}}

        def make_wo_job(b, j, tl, o, yts):
            def job():
                tags = ops_holder["tags"]
                slot = ops_holder["i"] % (2 * len(tags))
                tag = tags[slot // 2]
                if slot % 2 == 0:
                    ops_holder["tiles"][tag] = bank2(tag)
                ops = (ops_holder["tiles"][tag][:, 0:512] if slot % 2 == 0
                       else ops_holder["tiles"][tag][:, 512:1024])
                ops_holder["i"] += 1
                for h in range(QH):
                    nc.tensor.matmul(
                        ops, yts[h][:, 128 * tl:128 * (tl + 1)],
                        wo_sb[:, h, 512 * o:512 * (o + 1)],
                        start=h == 0, stop=h == QH - 1)
                ob = work.tile([128, 512], BF, tag="ob", bufs=6,
                               name="ob")
                if (tl + o) % 2 == 0:
                    nc.vector.tensor_copy(ob[:], ops)
                else:
                    nc.scalar.copy(ob[:], ops)
                r0 = b * T + 512 * j + 128 * tl
                nc.sync.dma_start(out_d[r0:r0 + 128, 512 * o:512 * (o + 1)],
                                  ob[:])
            return job

        for b in range(B):
            qT = act.tile([D, QH, T], BF, tag="qT", name="qT")
            kT = act.tile([D, T], BF, tag="kT", name="kT")
            vT = act.tile([D, T], BF, tag="vT", name="vT")
            vsb = act.tile([128, T // 128, D], BF, tag="v", name="vsb")

            # ---- projections ----
            # banks 0-1 = pq0,pq1; banks 2-3 = pq2,pq3; banks 4-5 = pk,pv
            for jc in range(NCH):
                bkA = bank2("bk01")
                bkB = bank2("bk23")
                bkC = bank2("bk45")
                pq = [bkA[:, 0:512], bkA[:, 512:1024],
                      bkB[:, 0:512], bkB[:, 512:1024]]
                pk = bkC[:, 0:512]
                pv = bkC[:, 512:1024]
                # q matmuls run SKEW c-tiles behind k/v so the previous
                # chunk's pq bank evictions are hidden behind ready work
                SKEW = 4
                xts = {}
                col0 = b * T + 512 * jc

                def q_mms(cq):
                    for h in range(QH):
                        nc.tensor.matmul(
                            pq[h], wq_sb[:, cq, 128 * h:128 * (h + 1)],
                            xts[cq][:], start=cq == 0, stop=cq == CT - 1)
                    if cq >= SKEW:
                        del xts[cq - SKEW]

                for ci in range(CT):
                    xt = work.tile([128, 512], BF, tag="xt", bufs=10, name="xt")
                    xts[ci] = xt
                    nc.sync.dma_start(
                        xt[:], xT_d[128 * ci:128 * (ci + 1), col0:col0 + 512])
                    st, sp = ci == 0, ci == CT - 1
                    nc.tensor.matmul(pk, wk_sb[:, ci, :], xt[:],
                                     start=st, stop=sp)
                    nc.tensor.matmul(pv, wv_sb[:, ci, :], xt[:],
                                     start=st, stop=sp)
                    if ci >= SKEW:
                        q_mms(ci - SKEW)
                for cq in range(CT - SKEW, CT):
                    q_mms(cq)
                cs = slice(512 * jc, 512 * (jc + 1))
                nc.scalar.copy(vT[:, cs], pv)
                last = jc == NCH - 1
                if last:
                    # transposes first (banks 6-7, free now) so attention can
                    # start while the last chunk's RoPE evictions trail
                    bkDv = bank2("bk67").bitcast(BF)
                    for k in range(T // 128):
                        tp = (bkDv[:, 0:128] if k % 2 == 0
                              else bkDv[:, 1024:1152])
                        nc.tensor.transpose(tp, vT[:, 128 * k:128 * (k + 1)],
                                            id_sb[:])
                        if k % 2 == 0:
                            nc.vector.tensor_copy(vsb[:, k, :], tp)
                        else:
                            nc.scalar.copy(vsb[:, k, :], tp)
                    rope_evict(qT[:, 0, cs], pq[0], cs)
                    rope_evict(qT[:, 1, cs], pq[1], cs)
                    rope_evict(kT[:, cs], pk, cs)
                    rope_evict(qT[:, 2, cs], pq[2], cs)
                    rope_evict(qT[:, 3, cs], pq[3], cs)
                else:
                    rope_evict(kT[:, cs], pk, cs)
                    for h in range(QH):
                        rope_evict(qT[:, h, cs], pq[h], cs)

            # ---- attention + output projection ----
            # banks 0-1 = yps,dps; banks 2-3 = wo accumulators; 4-7 = scores
            for j in range(NCH):
                yts = {}
                K = 4 * j + 4
                P = K // 2
                for h in range(QH):
                    # denominator pre-sum engine: DVE for the first/last head
                    # (short latency to the ones-matmul), GPSIMD for the rest
                    eng = nc.vector if h in (0, QH - 1) else nc.gpsimd
                    qs = qT[:, h, 512 * j:512 * (j + 1)]
                    # pass 1: paired score matmuls stream; paired exp trails
                    # on ACT; pair-level key pre-sum trails on DVE/GPSIMD
                    pts = []
                    accp = None
                    for p in range(P):
                        sps = bank2("bk67") if p % 2 == 0 else bank2("bk45")
                        nc.tensor.matmul(
                            sps[:, 0:512], kT[:, 256 * p:256 * p + 128],
                            qs, start=True, stop=True)
                        nc.tensor.matmul(
                            sps[:, 512:1024],
                            kT[:, 256 * p + 128:256 * p + 256],
                            qs, start=True, stop=True)
                        # pop PE jobs first: their PSUM evictions land ahead
                        # of this pair's exp-gated ops in the engine FIFOs
                        if wo_jobs:
                            wo_jobs.popleft()()
                        if wo_jobs:
                            wo_jobs.popleft()()
                        pt = work.tile([128, 1024], BF, tag="pt", bufs=10,
                                       name="pt")
                        nc.scalar.activation(pt[:], sps[:], AFT.Exp,
                                             bias=bias_sb[:], scale=SCALE)
                        o = 2 * p - 4 * j
                        if o >= 0:
                            nc.vector.tensor_mul(pt[:], pt[:],
                                                 alw_sb[:, o // 2, :])
                        if p == 1:
                            accp = work.tile([128, 1024], F32, tag="accp",
                                             bufs=4, name="accp")
                            eng.tensor_add(accp[:], pts[0][:], pt[:])
                        elif p > 1:
                            eng.tensor_add(accp[:], accp[:], pt[:])
                        pts.append(pt)
                    # pass 2: attn@v accumulation (dense PE)
                    bkY = bank2("bk01")
                    yps = bkY[:, 0:512]
                    dps = bkY[:, 512:1024]
                    for k in range(K):
                        nc.tensor.matmul(
                            yps, vsb[:, k, :],
                            pts[k // 2][:, 512 * (k % 2):512 * (k % 2) + 512],
                            start=(k == 0), stop=(k == K - 1))
                    # fold pair-accumulator; dense PE jobs cover the latency
                    accb = work.tile([128, 512], BF, tag="accb", bufs=4,
                                     name="accb")
                    eng.tensor_add(accb[:], accp[:, 0:512], accp[:, 512:1024])
                    for _ in range(3):
                        if wo_jobs:
                            wo_jobs.popleft()()
                    nc.tensor.matmul(dps, onesbf_sb[:], accb[:],
                                     start=True, stop=True)
                    rec = work.tile([128, 512], F32, tag="rec", bufs=2,
                                    name="rec")
                    nc.vector.reciprocal_approx_fast(rec[:], dps)
                    yt = work.tile([128, 512], BF, tag="yt", bufs=8,
                                   name="yt")
                    nc.vector.tensor_mul(yt[:], yps, rec[:])
                    yts[h] = yt
                for tl in range(4):
                    for o in range(C // 512):
                        wo_jobs.append(make_wo_job(b, j, tl, o, yts))
            # keep a few jobs alive across the batch boundary so the next
            # batch's first attention chunk has dense PE filler work
            keep = 12 if b < B - 1 else 0
            ops_holder["tags"] = ["bk23", "bk45", "bk67"]
            ops_holder["i"] = 0
            while len(wo_jobs) > keep:
                wo_jobs.popleft()()
            ops_holder["tags"] = ["bk23"]
            ops_holder["i"] = 0

    nc.compile()
    return nc


def host_prep(inputs):
    x = np.asarray(inputs["x"], np.float32)
    mask = np.asarray(inputs["mask"], np.float32)
    wq = np.asarray(inputs["wq"], np.float32)
    wk = np.asarray(inputs["wk"], np.float32)
    wv = np.asarray(inputs["wv"], np.float32)
    wo = np.asarray(inputs["wo"], np.float32)

    xT = np.ascontiguousarray(x.reshape(B * T, C).T).astype(bf16)
    inv = 1.0 / (ROPE_BASE ** (np.arange(0, D, 2, dtype=np.float64) / D))
    freqs = np.arange(T, dtype=np.float64)[:, None] * inv[None, :] * B
    emb = np.concatenate([freqs, freqs], axis=-1)       # [T, D]
    cosT = np.cos(emb).T.astype(np.float32).astype(bf16)
    sinT = np.sin(emb).T.astype(np.float32)
    sinT[: D // 2] *= -1.0
    sinTr = sinT.astype(bf16)
    # allow[p, o, jj] = 1 - mask[jj, 128*o + p]  (from the actual mask input),
    # stored as two k-tile PAIRS so one DVE mul masks a whole [128,1024] pair
    allowA = np.stack([(1.0 - mask[0:512, 128 * o:128 * (o + 1)]).T
                       for o in range(4)], axis=1)            # [128, 4, 512]
    allowP = np.ascontiguousarray(
        allowA.reshape(128, 2, 1024)).astype(bf16)            # [128, 2, 1024]
    ident = np.eye(128, dtype=np.float32).astype(bf16)

    common = dict(xT=xT, cosT=cosT, sinTr=sinTr, allowP=allowP, ident=ident)
    in_maps = []
    for c in range(NCORES):
        m = dict(common)
        m["wq"] = np.ascontiguousarray(wq[:, 512 * c:512 * (c + 1)]).astype(bf16)
        m["wk"] = np.ascontiguousarray(wk[:, 128 * c:128 * (c + 1)]).astype(bf16)
        m["wv"] = np.ascontiguousarray(wv[:, 128 * c:128 * (c + 1)]).astype(bf16)
        m["woA"] = np.ascontiguousarray(
            wo[512 * c:512 * (c + 1), :].reshape(QH, 128, C)
            .transpose(1, 0, 2)).astype(bf16)
        in_maps.append(m)
    return in_maps


def kernel(**inputs) -> np.ndarray:
    from concourse.bass_utils import run_bass_kernel_spmd

    in_maps = host_prep(inputs)
    nc = emit_program()
    trace = bool(os.environ.get("BASS_KERNEL_TRACE"))
    res = run_bass_kernel_spmd(nc, in_maps, core_ids=list(range(NCORES)),
                               trace=trace)
    if trace and res.exec_time_ns is not None:
        print(f"HW exec time: {res.exec_time_ns} ns")
        if res.instructions_and_trace is not None:
            print("trace:", res.instructions_and_trace[1])
    total = np.zeros((B * T, C), np.float32)
    for r in res.results:
        total += np.asarray(r["out"], np.float32)
    return total.reshape(B, T, C)


# revision 24
# speedup vs baseline: 1.0745x; 1.0745x over previous
"""Trainium2 Bass kernel for GQA attention (B=2, T=2048, C=4096, H=32, KV=8, D=128)
with RoPE and causal mask.

Sharding: tensor-parallel over heads across 8 cores. Each core owns 4 Q heads and
their shared KV head: projects q/k/v for those heads, runs causal attention, and
computes a partial output projection; the host sums the 8 partials (bf16).

All on-chip layouts are transposed ([feature, token]) so every matmul consumes
natural slices:
  qT/kT/vT = W^T @ x  via lhsT=W-tile [128c, cols], rhs=xT-tile [128c, 512t]
  sT[tk, tq] = kT-tile^T @ qT-chunk, two k-tiles paired into one [128,1024]
  PSUM tile so ACT computes exp on both halves in one ACTIVATE
  pT = exp(sT/sqrt(D) - 10); strictly-causal-upper tiles skipped entirely
  yT[d, tq] += v-tile^T @ pT          (v pre-transposed to [t, d] via PE transpose)
  out[tq, :] += yT_h^T @ wo_h         (accumulate 4 heads in PSUM, evict bf16)

Softmax denominator: the exp tiles are pre-summed across key-tiles on DVE
(heads 0,2) / GPSIMD (heads 1,3) into a bf16 row-sum; a single ones-matmul per
(head, chunk) reduces it across partitions. This removes the per-key-tile
ones-matmul stream (~9% of PE cycles) from the tensor engine.

PSUM is managed as ONE kernel-wide pool of four 2-bank [128,1024] tags so no
pool-release barrier ever serializes phase transitions (per-address WAR deps
only): banks 0-1 = pq0/pq1 then yps/dps; banks 2-3 = pq2/pq3 then the wo-job
accumulators; banks 4-5 = pk/pv then odd score pairs; banks 6-7 = V-transpose
ping-pong then even score pairs. At the end of each batch's projections the
V transposes + their evictions are emitted BEFORE the last chunk's RoPE
evictions, so the first attention scores (banks 6-7) start within ~4us of the
last projection matmul and the PE never goes HAM-cold at the phase boundary.
Output-projection matmul "jobs" are popped from a queue inside the score
streams to keep the in-order PE queue dense while ACT works through the exps.
"""

import os
from collections import deque
from contextlib import ExitStack

import numpy as np
import ml_dtypes

import concourse.bacc as bacc
import concourse.mybir as mybir
import concourse.tile as tile

BF = mybir.dt.bfloat16
F32 = mybir.dt.float32
AFT = mybir.ActivationFunctionType

NCORES = 8
B, T, C = 2, 2048, 4096
H, KV, D = 32, 8, 128
QH = H // NCORES          # 4 q-heads per core
CT = C // 128             # 32 contraction tiles
NCH = T // 512            # 4 query chunks per batch
SCALE = 1.0 / float(np.sqrt(D))
EXP_BIAS = -10.0
ROPE_BASE = 10000.0

bf16 = ml_dtypes.bfloat16


def emit_program():
    nc = bacc.Bacc("TRN2", target_bir_lowering=False, debug=False,
                   num_devices=NCORES)

    xT_d = nc.dram_tensor("xT", [C, B * T], BF, kind="ExternalInput").ap()
    wq_d = nc.dram_tensor("wq", [C, QH * D], BF, kind="ExternalInput").ap()
    wk_d = nc.dram_tensor("wk", [C, D], BF, kind="ExternalInput").ap()
    wv_d = nc.dram_tensor("wv", [C, D], BF, kind="ExternalInput").ap()
    wo_d = nc.dram_tensor("woA", [128, QH, C], BF, kind="ExternalInput").ap()
    cos_d = nc.dram_tensor("cosT", [D, T], BF, kind="ExternalInput").ap()
    sin_d = nc.dram_tensor("sinTr", [D, T], BF, kind="ExternalInput").ap()
    alw_d = nc.dram_tensor("negP", [128, 2, 1024], BF, kind="ExternalInput").ap()
    id_d = nc.dram_tensor("ident", [128, 128], BF, kind="ExternalInput").ap()
    out_d = nc.dram_tensor("out", [B * T, C], BF, kind="ExternalOutput").ap()

    with tile.TileContext(nc) as tc, ExitStack() as ctx:
        const = ctx.enter_context(tc.tile_pool(name="const", bufs=1))
        act = ctx.enter_context(tc.tile_pool(name="act", bufs=1))
        work = ctx.enter_context(tc.tile_pool(name="work", bufs=1))
        ps = ctx.enter_context(tc.tile_pool(name="ps", bufs=1, space="PSUM"))

        def bank2(tag):
            return ps.tile([128, 1024], F32, tag=tag, bufs=1, name=tag)

        # weights + tables on the gpsimd DMA queue so they never sit ahead of
        # the xt activation loads (sync queue); chunked in 8-c-tile groups so
        # the first projection matmuls wait on ~1MB, not the full tensors
        wq_sb = const.tile([128, CT, QH * D], BF)
        wk_sb = const.tile([128, CT, D], BF)
        wv_sb = const.tile([128, CT, D], BF)
        xTr = xT_d.rearrange("(ci p) t -> p ci t", p=128)
        wqr = wq_d.rearrange("(ci p) n -> p ci n", p=128)
        wkr = wk_d.rearrange("(ci p) n -> p ci n", p=128)
        wvr = wv_d.rearrange("(ci p) n -> p ci n", p=128)
        GW = 8
        for g in range(0, CT, GW):
            s = slice(g, g + GW)
            nc.gpsimd.dma_start(wq_sb[:, s, :], wqr[:, s, :])
            nc.gpsimd.dma_start(wk_sb[:, s, :], wkr[:, s, :])
            nc.gpsimd.dma_start(wv_sb[:, s, :], wvr[:, s, :])
        cos_sb = const.tile([D, T], BF)
        nc.gpsimd.dma_start(cos_sb[:], cos_d)
        sin_sb = const.tile([D, T], BF)
        nc.gpsimd.dma_start(sin_sb[:], sin_d)
        alw_sb = const.tile([128, 2, 1024], BF)
        nc.gpsimd.dma_start(alw_sb[:], alw_d)
        id_sb = const.tile([128, 128], BF)
        nc.gpsimd.dma_start(id_sb[:], id_d)
        wo_sb = const.tile([128, QH, C], BF)
        nc.gpsimd.dma_start(wo_sb[:], wo_d)
        onesbf_sb = const.tile([128, 128], BF)
        nc.gpsimd.memset(onesbf_sb[:], 1.0)
        bias_sb = const.tile([128, 1], F32)
        nc.gpsimd.memset(bias_sb[:], EXP_BIAS)

        def rope_evict(dst, psum, cs):
            # dst = psum * cos + swap_halves(psum) * sin_rot   (bf16 out)
            # half-swap copies on ACT (fast PSUM reads), muls/add on DVE
            sw = work.tile([128, 512], F32, tag="sw", bufs=3, name="sw")
            nc.scalar.copy(sw[0:64, :], psum[64:128, :])
            nc.scalar.copy(sw[64:128, :], psum[0:64, :])
            nc.vector.tensor_mul(sw[:], sw[:], sin_sb[:, cs])
            cst = work.tile([128, 512], F32, tag="cst", bufs=3, name="cst")
            nc.vector.tensor_mul(cst[:], psum[:], cos_sb[:, cs])
            nc.vector.tensor_add(dst, cst[:], sw[:])

        wo_jobs = deque()
        # wo-job PSUM accumulators ping-pong over half-bank slots; during the
        # end-of-batch drain the idle score banks join the rotation so PE
        # never waits for an eviction
        ops_holder = {"tags": ["bk23"], "i": 0, "tiles": {}}

        def make_wo_job(b, j, tl, o, yts):
            def job():
                tags = ops_holder["tags"]
                slot = ops_holder["i"] % (2 * len(tags))
                tag = tags[slot // 2]
                if slot % 2 == 0:
                    ops_holder["tiles"][tag] = bank2(tag)
                ops = (ops_holder["tiles"][tag][:, 0:512] if slot % 2 == 0
                       else ops_holder["tiles"][tag][:, 512:1024])
                ops_holder["i"] += 1
                for h in range(QH):
                    nc.tensor.matmul(
                        ops, yts[h][:, 128 * tl:128 * (tl + 1)],
                        wo_sb[:, h, 512 * o:512 * (o + 1)],
                        start=h == 0, stop=h == QH - 1)
                ob = work.tile([128, 512], BF, tag="ob", bufs=6,
                               name="ob")
                if (tl + o) % 2 == 0:
                    nc.vector.tensor_copy(ob[:], ops)
                else:
                    nc.scalar.copy(ob[:], ops)
                r0 = b * T + 512 * j + 128 * tl
                nc.sync.dma_start(out_d[r0:r0 + 128, 512 * o:512 * (o + 1)],
                                  ob[:])
            return job

        for b in range(B):
            qT = act.tile([D, QH, T], BF, tag="qT", name="qT")
            kT = act.tile([D, T], BF, tag="kT", name="kT")
            vT = act.tile([D, T], BF, tag="vT", name="vT")
            vsb = act.tile([128, T // 128, D], BF, tag="v", name="vsb")

            # ---- projections ----
            # banks 0-1 = pq0,pq1; banks 2-3 = pq2,pq3; banks 4-5 = pk,pv
            for jc in range(NCH):
                bkA = bank2("bk01")
                bkB = bank2("bk23")
                bkC = bank2("bk45")
                pq = [bkA[:, 0:512], bkA[:, 512:1024],
                      bkB[:, 0:512], bkB[:, 512:1024]]
                pk = bkC[:, 0:512]
                pv = bkC[:, 512:1024]
                # q matmuls run SKEW c-tiles behind k/v so the previous
                # chunk's pq bank evictions are hidden behind ready work
                SKEW = 4
                xts = {}
                col0 = b * T + 512 * jc

                def q_mms(cq):
                    for h in range(QH):
                        nc.tensor.matmul(
                            pq[h], wq_sb[:, cq, 128 * h:128 * (h + 1)],
                            xts[cq], start=cq == 0, stop=cq == CT - 1)
                    if cq >= SKEW:
                        del xts[cq - SKEW]

                for ci in range(CT):
                    if ci % 2 == 0:
                        # one DMA covers two contraction tiles (fewer, larger
                        # transfers keep the activation stream ahead of PE)
                        xt2 = work.tile([128, 2, 512], BF, tag="xt", bufs=6,
                                        name="xt2")
                        nc.sync.dma_start(
                            xt2[:], xTr[:, ci:ci + 2, col0:col0 + 512])
                        xts[ci] = xt2[:, 0, :]
                        xts[ci + 1] = xt2[:, 1, :]
                    st, sp = ci == 0, ci == CT - 1
                    nc.tensor.matmul(pk, wk_sb[:, ci, :], xts[ci],
                                     start=st, stop=sp)
                    nc.tensor.matmul(pv, wv_sb[:, ci, :], xts[ci],
                                     start=st, stop=sp)
                    if ci >= SKEW:
                        q_mms(ci - SKEW)
                for cq in range(CT - SKEW, CT):
                    q_mms(cq)
                cs = slice(512 * jc, 512 * (jc + 1))
                nc.scalar.copy(vT[:, cs], pv)
                last = jc == NCH - 1
                if last:
                    # transposes first (banks 6-7, free now) so attention can
                    # start while the last chunk's RoPE evictions trail
                    bkDv = bank2("bk67").bitcast(BF)
                    for k in range(T // 128):
                        tp = (bkDv[:, 0:128] if k % 2 == 0
                              else bkDv[:, 1024:1152])
                        nc.tensor.transpose(tp, vT[:, 128 * k:128 * (k + 1)],
                                            id_sb[:])
                        if k % 2 == 0:
                            nc.vector.tensor_copy(vsb[:, k, :], tp)
                        else:
                            nc.scalar.copy(vsb[:, k, :], tp)
                    rope_evict(qT[:, 0, cs], pq[0], cs)
                    rope_evict(qT[:, 1, cs], pq[1], cs)
                    rope_evict(kT[:, cs], pk, cs)
                    rope_evict(qT[:, 2, cs], pq[2], cs)
                    rope_evict(qT[:, 3, cs], pq[3], cs)
                else:
                    rope_evict(kT[:, cs], pk, cs)
                    for h in range(QH):
                        rope_evict(qT[:, h, cs], pq[h], cs)

            # ---- attention + output projection ----
            # banks 0-1 = yps,dps; banks 2-3 = wo accumulators; 4-7 = scores
            for j in range(NCH):
                yts = {}
                K = 4 * j + 4
                P = K // 2
                for h in range(QH):
                    # denominator pre-sum engine: DVE for the first/last head
                    # (short latency to the ones-matmul), GPSIMD for the rest
                    eng = nc.vector if h in (0, QH - 1) else nc.gpsimd
                    qs = qT[:, h, 512 * j:512 * (j + 1)]
                    # pass 1: paired score matmuls stream; paired exp trails
                    # on ACT; pair-level key pre-sum trails on DVE/GPSIMD
                    pts = []
                    accp = None
                    for p in range(P):
                        sps = bank2("bk67") if p % 2 == 0 else bank2("bk45")
                        nc.tensor.matmul(
                            sps[:, 0:512], kT[:, 256 * p:256 * p + 128],
                            qs, start=True, stop=True)
                        nc.tensor.matmul(
                            sps[:, 512:1024],
                            kT[:, 256 * p + 128:256 * p + 256],
                            qs, start=True, stop=True)
                        # additive causal mask applied to the PSUM scores
                        # (cheap DVE PSUM op; keeps the post-exp path clean)
                        o = 2 * p - 4 * j
                        if o >= 0:
                            nc.vector.tensor_add(sps[:], sps[:],
                                                 alw_sb[:, o // 2, :])
                        # pop PE jobs next: their PSUM evictions land ahead
                        # of this pair's exp-gated ops in the engine FIFOs
                        if wo_jobs:
                            wo_jobs.popleft()()
                        if wo_jobs:
                            wo_jobs.popleft()()
                        pt = work.tile([128, 1024], BF, tag="pt", bufs=10,
                                       name="pt")
                        nc.scalar.activation(pt[:], sps[:], AFT.Exp,
                                             bias=bias_sb[:], scale=SCALE)
                        if p == 1:
                            accp = work.tile([128, 1024], F32, tag="accp",
                                             bufs=4, name="accp")
                            eng.tensor_add(accp[:], pts[0][:], pt[:])
                        elif p > 1:
                            eng.tensor_add(accp[:], accp[:], pt[:])
                        pts.append(pt)
                    # pass 2: attn@v accumulation (dense PE)
                    bkY = bank2("bk01")
                    yps = bkY[:, 0:512]
                    dps = bkY[:, 512:1024]
                    for k in range(K):
                        nc.tensor.matmul(
                            yps, vsb[:, k, :],
                            pts[k // 2][:, 512 * (k % 2):512 * (k % 2) + 512],
                            start=(k == 0), stop=(k == K - 1))
                    # fold pair-accumulator; dense PE jobs cover the latency
                    accb = work.tile([128, 512], BF, tag="accb", bufs=4,
                                     name="accb")
                    eng.tensor_add(accb[:], accp[:, 0:512], accp[:, 512:1024])
                    for _ in range(3):
                        if wo_jobs:
                            wo_jobs.popleft()()
                    nc.tensor.matmul(dps, onesbf_sb[:], accb[:],
                                     start=True, stop=True)
                    rec = work.tile([128, 512], F32, tag="rec", bufs=2,
                                    name="rec")
                    nc.vector.reciprocal_approx_fast(rec[:], dps)
                    yt = work.tile([128, 512], BF, tag="yt", bufs=8,
                                   name="yt")
                    nc.vector.tensor_mul(yt[:], yps, rec[:])
                    yts[h] = yt
                for tl in range(4):
                    for o in range(C // 512):
                        wo_jobs.append(make_wo_job(b, j, tl, o, yts))
            # keep a few jobs alive across the batch boundary so the next
            # batch's first attention chunk has dense PE filler work
            keep = 12 if b < B - 1 else 0
            ops_holder["tags"] = ["bk23", "bk45", "bk67"]
            ops_holder["i"] = 0
            while len(wo_jobs) > keep:
                wo_jobs.popleft()()
            ops_holder["tags"] = ["bk23"]
            ops_holder["i"] = 0

    nc.compile()
    return nc


def host_prep(inputs):
    x = np.asarray(inputs["x"], np.float32)
    mask = np.asarray(inputs["mask"], np.float32)
    wq = np.asarray(inputs["wq"], np.float32)
    wk = np.asarray(inputs["wk"], np.float32)
    wv = np.asarray(inputs["wv"], np.float32)
    wo = np.asarray(inputs["wo"], np.float32)

    xT = np.ascontiguousarray(x.reshape(B * T, C).T).astype(bf16)
    inv = 1.0 / (ROPE_BASE ** (np.arange(0, D, 2, dtype=np.float64) / D))
    freqs = np.arange(T, dtype=np.float64)[:, None] * inv[None, :] * B
    emb = np.concatenate([freqs, freqs], axis=-1)       # [T, D]
    cosT = np.cos(emb).T.astype(np.float32).astype(bf16)
    sinT = np.sin(emb).T.astype(np.float32)
    sinT[: D // 2] *= -1.0
    sinTr = sinT.astype(bf16)
    # additive causal mask: -30000 where mask[jj, 128*o + p] says "disallow",
    # 0 elsewhere; stored as two k-tile PAIRS so one DVE add masks a whole
    # [128,1024] PSUM score pair before the exp
    allowA = np.stack([(1.0 - mask[0:512, 128 * o:128 * (o + 1)]).T
                       for o in range(4)], axis=1)            # [128, 4, 512]
    negP = np.ascontiguousarray(
        (allowA.reshape(128, 2, 1024) - 1.0) * 30000.0).astype(bf16)
    ident = np.eye(128, dtype=np.float32).astype(bf16)

    common = dict(xT=xT, cosT=cosT, sinTr=sinTr, negP=negP, ident=ident)
    in_maps = []
    for c in range(NCORES):
        m = dict(common)
        m["wq"] = np.ascontiguousarray(wq[:, 512 * c:512 * (c + 1)]).astype(bf16)
        m["wk"] = np.ascontiguousarray(wk[:, 128 * c:128 * (c + 1)]).astype(bf16)
        m["wv"] = np.ascontiguousarray(wv[:, 128 * c:128 * (c + 1)]).astype(bf16)
        m["woA"] = np.ascontiguousarray(
            wo[512 * c:512 * (c + 1), :].reshape(QH, 128, C)
            .transpose(1, 0, 2)).astype(bf16)
        in_maps.append(m)
    return in_maps


def kernel(**inputs) -> np.ndarray:
    from concourse.bass_utils import run_bass_kernel_spmd

    in_maps = host_prep(inputs)
    nc = emit_program()
    trace = bool(os.environ.get("BASS_KERNEL_TRACE"))
    res = run_bass_kernel_spmd(nc, in_maps, core_ids=list(range(NCORES)),
                               trace=trace)
    if trace and res.exec_time_ns is not None:
        print(f"HW exec time: {res.exec_time_ns} ns")
        if res.instructions_and_trace is not None:
            print("trace:", res.instructions_and_trace[1])
    total = np.zeros((B * T, C), np.float32)
    for r in res.results:
        total += np.asarray(r["out"], np.float32)
    return total.reshape(B, T, C)


# revision 27
# speedup vs baseline: 1.0837x; 1.0086x over previous
"""Trainium2 Bass kernel for GQA attention (B=2, T=2048, C=4096, H=32, KV=8, D=128)
with RoPE and causal mask.

Sharding: tensor-parallel over heads across 8 cores. Each core owns 4 Q heads and
their shared KV head: projects q/k/v for those heads, runs causal attention, and
computes a partial output projection; the host sums the 8 partials (bf16).

All on-chip layouts are transposed ([feature, token]) so every matmul consumes
natural slices:
  qT/kT/vT = W^T @ x  via lhsT=W-tile [128c, cols], rhs=xT-tile [128c, 512t]
  sT[tk, tq] = kT-tile^T @ qT-chunk, two k-tiles paired into one [128,1024]
  PSUM tile so ACT computes exp on both halves in one ACTIVATE
  pT = exp(sT/sqrt(D) - 10); strictly-causal-upper tiles skipped entirely
  yT[d, tq] += v-tile^T @ pT          (v pre-transposed to [t, d] via PE transpose)
  out[tq, :] += yT_h^T @ wo_h         (accumulate 4 heads in PSUM, evict bf16)

Softmax denominator: the exp tiles are pre-summed across key-tiles on DVE
(heads 0,2) / GPSIMD (heads 1,3) into a bf16 row-sum; a single ones-matmul per
(head, chunk) reduces it across partitions. This removes the per-key-tile
ones-matmul stream (~9% of PE cycles) from the tensor engine.

PSUM is managed as ONE kernel-wide pool of four 2-bank [128,1024] tags so no
pool-release barrier ever serializes phase transitions (per-address WAR deps
only): banks 0-1 = pq0/pq1 then yps/dps; banks 2-3 = pq2/pq3 then the wo-job
accumulators; banks 4-5 = pk/pv then odd score pairs; banks 6-7 = V-transpose
ping-pong then even score pairs. At the end of each batch's projections the
V transposes + their evictions are emitted BEFORE the last chunk's RoPE
evictions, so the first attention scores (banks 6-7) start within ~4us of the
last projection matmul and the PE never goes HAM-cold at the phase boundary.
Output-projection matmul "jobs" are popped from a queue inside the score
streams to keep the in-order PE queue dense while ACT works through the exps.
"""

import os
from collections import deque
from contextlib import ExitStack

import numpy as np
import ml_dtypes

import concourse.bacc as bacc
import concourse.mybir as mybir
import concourse.tile as tile

BF = mybir.dt.bfloat16
F32 = mybir.dt.float32
AFT = mybir.ActivationFunctionType

NCORES = 8
B, T, C = 2, 2048, 4096
H, KV, D = 32, 8, 128
QH = H // NCORES          # 4 q-heads per core
CT = C // 128             # 32 contraction tiles
NCH = T // 512            # 4 query chunks per batch
SCALE = 1.0 / float(np.sqrt(D))
EXP_BIAS = -10.0
ROPE_BASE = 10000.0

bf16 = ml_dtypes.bfloat16


def emit_program():
    nc = bacc.Bacc("TRN2", target_bir_lowering=False, debug=False,
                   num_devices=NCORES)

    xT_d = nc.dram_tensor("xT", [C, B * T], BF, kind="ExternalInput").ap()
    wq_d = nc.dram_tensor("wq", [C, QH * D], BF, kind="ExternalInput").ap()
    wk_d = nc.dram_tensor("wk", [C, D], BF, kind="ExternalInput").ap()
    wv_d = nc.dram_tensor("wv", [C, D], BF, kind="ExternalInput").ap()
    wo_d = nc.dram_tensor("woA", [128, QH, C], BF, kind="ExternalInput").ap()
    cos_d = nc.dram_tensor("cosT", [D, T], BF, kind="ExternalInput").ap()
    sin_d = nc.dram_tensor("sinTr", [D, T], BF, kind="ExternalInput").ap()
    alw_d = nc.dram_tensor("negP", [128, 2, 1024], BF, kind="ExternalInput").ap()
    id_d = nc.dram_tensor("ident", [128, 128], BF, kind="ExternalInput").ap()
    out_d = nc.dram_tensor("out", [B * T, C], BF, kind="ExternalOutput").ap()

    with tile.TileContext(nc) as tc, ExitStack() as ctx:
        const = ctx.enter_context(tc.tile_pool(name="const", bufs=1))
        act = ctx.enter_context(tc.tile_pool(name="act", bufs=1))
        work = ctx.enter_context(tc.tile_pool(name="work", bufs=1))
        ps = ctx.enter_context(tc.tile_pool(name="ps", bufs=1, space="PSUM"))

        def bank2(tag):
            return ps.tile([128, 1024], F32, tag=tag, bufs=1, name=tag)

        # weights + tables on the gpsimd DMA queue so they never sit ahead of
        # the xt activation loads (sync queue); chunked in 8-c-tile groups so
        # the first projection matmuls wait on ~1MB, not the full tensors
        wq_sb = const.tile([128, CT, QH * D], BF)
        wk_sb = const.tile([128, CT, D], BF)
        wv_sb = const.tile([128, CT, D], BF)
        xTr = xT_d.rearrange("(ci p) t -> p ci t", p=128)
        wqr = wq_d.rearrange("(ci p) n -> p ci n", p=128)
        wkr = wk_d.rearrange("(ci p) n -> p ci n", p=128)
        wvr = wv_d.rearrange("(ci p) n -> p ci n", p=128)
        GW = 8
        for g in range(0, CT, GW):
            s = slice(g, g + GW)
            nc.gpsimd.dma_start(wq_sb[:, s, :], wqr[:, s, :])
            nc.gpsimd.dma_start(wk_sb[:, s, :], wkr[:, s, :])
            nc.gpsimd.dma_start(wv_sb[:, s, :], wvr[:, s, :])
        cos_sb = const.tile([D, T], BF)
        nc.gpsimd.dma_start(cos_sb[:], cos_d)
        sin_sb = const.tile([D, T], BF)
        nc.gpsimd.dma_start(sin_sb[:], sin_d)
        alw_sb = const.tile([128, 2, 1024], BF)
        nc.gpsimd.dma_start(alw_sb[:], alw_d)
        id_sb = const.tile([128, 128], BF)
        nc.gpsimd.dma_start(id_sb[:], id_d)
        wo_sb = const.tile([128, QH, C], BF)
        nc.gpsimd.dma_start(wo_sb[:], wo_d)
        onesbf_sb = const.tile([128, 128], BF)
        nc.gpsimd.memset(onesbf_sb[:], 1.0)
        bias_sb = const.tile([128, 1], F32)
        nc.gpsimd.memset(bias_sb[:], EXP_BIAS)

        def rope_evict(dst, psum, cs):
            # dst = psum * cos + swap_halves(psum) * sin_rot   (bf16 out)
            # half-swap copies on ACT (fast PSUM reads), muls/add on DVE
            sw = work.tile([128, 512], F32, tag="sw", bufs=3, name="sw")
            nc.scalar.copy(sw[0:64, :], psum[64:128, :])
            nc.scalar.copy(sw[64:128, :], psum[0:64, :])
            nc.vector.tensor_mul(sw[:], sw[:], sin_sb[:, cs])
            cst = work.tile([128, 512], F32, tag="cst", bufs=3, name="cst")
            nc.vector.tensor_mul(cst[:], psum[:], cos_sb[:, cs])
            nc.vector.tensor_add(dst, cst[:], sw[:])

        wo_jobs = deque()
        # wo-job PSUM accumulators ping-pong over half-bank slots; during the
        # end-of-batch drain the idle score banks join the rotation so PE
        # never waits for an eviction
        ops_holder = {"tags": ["bk23"], "i": 0, "tiles": {}}

        def make_wo_job(b, j, tl, o, yts):
            def job():
                tags = ops_holder["tags"]
                slot = ops_holder["i"] % (2 * len(tags))
                tag = tags[slot // 2]
                if slot % 2 == 0:
                    ops_holder["tiles"][tag] = bank2(tag)
                ops = (ops_holder["tiles"][tag][:, 0:512] if slot % 2 == 0
                       else ops_holder["tiles"][tag][:, 512:1024])
                ops_holder["i"] += 1
                for h in range(QH):
                    nc.tensor.matmul(
                        ops, yts[h][:, 128 * tl:128 * (tl + 1)],
                        wo_sb[:, h, 512 * o:512 * (o + 1)],
                        start=h == 0, stop=h == QH - 1)
                ob = work.tile([128, 512], BF, tag="ob", bufs=6,
                               name="ob")
                if (tl + o) % 2 == 0:
                    nc.vector.tensor_copy(ob[:], ops)
                else:
                    nc.scalar.copy(ob[:], ops)
                r0 = b * T + 512 * j + 128 * tl
                nc.sync.dma_start(out_d[r0:r0 + 128, 512 * o:512 * (o + 1)],
                                  ob[:])
            return job

        for b in range(B):
            qT = act.tile([D, QH, T], BF, tag="qT", name="qT")
            kT = act.tile([D, T], BF, tag="kT", name="kT")
            vT = act.tile([D, T], BF, tag="vT", name="vT")
            vsb = act.tile([128, T // 128, D], BF, tag="v", name="vsb")

            # ---- projections ----
            # banks 0-1 = pq0,pq1; banks 2-3 = pq2,pq3; banks 4-5 = pk,pv
            for jc in range(NCH):
                bkA = bank2("bk01")
                bkB = bank2("bk23")
                bkC = bank2("bk45")
                pq = [bkA[:, 0:512], bkA[:, 512:1024],
                      bkB[:, 0:512], bkB[:, 512:1024]]
                pk = bkC[:, 0:512]
                pv = bkC[:, 512:1024]
                # q matmuls run SKEW c-tiles behind k/v so the previous
                # chunk's pq bank evictions are hidden behind ready work
                SKEW = 4
                xts = {}
                col0 = b * T + 512 * jc

                def q_mms(cq):
                    for h in range(QH):
                        nc.tensor.matmul(
                            pq[h], wq_sb[:, cq, 128 * h:128 * (h + 1)],
                            xts[cq], start=cq == 0, stop=cq == CT - 1)
                    if cq >= SKEW:
                        del xts[cq - SKEW]

                for ci in range(CT):
                    if ci % 2 == 0:
                        # one DMA covers two contraction tiles (fewer, larger
                        # transfers keep the activation stream ahead of PE)
                        xt2 = work.tile([128, 2, 512], BF, tag="xt", bufs=6,
                                        name="xt2")
                        nc.sync.dma_start(
                            xt2[:], xTr[:, ci:ci + 2, col0:col0 + 512])
                        xts[ci] = xt2[:, 0, :]
                        xts[ci + 1] = xt2[:, 1, :]
                    st, sp = ci == 0, ci == CT - 1
                    nc.tensor.matmul(pk, wk_sb[:, ci, :], xts[ci],
                                     start=st, stop=sp)
                    nc.tensor.matmul(pv, wv_sb[:, ci, :], xts[ci],
                                     start=st, stop=sp)
                    if ci >= SKEW:
                        q_mms(ci - SKEW)
                for cq in range(CT - SKEW, CT):
                    q_mms(cq)
                cs = slice(512 * jc, 512 * (jc + 1))
                nc.scalar.copy(vT[:, cs], pv)
                last = jc == NCH - 1
                if last:
                    # transposes first (banks 6-7, free now) so attention can
                    # start while the last chunk's RoPE evictions trail
                    bkDv = bank2("bk67").bitcast(BF)
                    for k in range(T // 128):
                        tp = (bkDv[:, 0:128] if k % 2 == 0
                              else bkDv[:, 1024:1152])
                        nc.tensor.transpose(tp, vT[:, 128 * k:128 * (k + 1)],
                                            id_sb[:])
                        if k % 2 == 0:
                            nc.vector.tensor_copy(vsb[:, k, :], tp)
                        else:
                            nc.scalar.copy(vsb[:, k, :], tp)
                    rope_evict(qT[:, 0, cs], pq[0], cs)
                    rope_evict(qT[:, 1, cs], pq[1], cs)
                    rope_evict(kT[:, cs], pk, cs)
                    rope_evict(qT[:, 2, cs], pq[2], cs)
                    rope_evict(qT[:, 3, cs], pq[3], cs)
                else:
                    rope_evict(kT[:, cs], pk, cs)
                    for h in range(QH):
                        rope_evict(qT[:, h, cs], pq[h], cs)

            # ---- attention + output projection ----
            # banks 0-1 = yps,dps; banks 2-3 = wo accumulators; 4-7 = scores
            for j in range(NCH):
                yts = {}
                K = 4 * j + 4
                P = K // 2
                for h in range(QH):
                    # denominator pre-sum engine: DVE for the first/last head
                    # (short latency to the ones-matmul), GPSIMD for the rest
                    eng = nc.vector if h in (0, QH - 1) else nc.gpsimd
                    qs = qT[:, h, 512 * j:512 * (j + 1)]
                    # pass 1: paired score matmuls stream; paired exp trails
                    # on ACT; pair-level key pre-sum trails on DVE/GPSIMD
                    pts = []
                    accp = None
                    for p in range(P):
                        sps = bank2("bk67") if p % 2 == 0 else bank2("bk45")
                        nc.tensor.matmul(
                            sps[:, 0:512], kT[:, 256 * p:256 * p + 128],
                            qs, start=True, stop=True)
                        nc.tensor.matmul(
                            sps[:, 512:1024],
                            kT[:, 256 * p + 128:256 * p + 256],
                            qs, start=True, stop=True)
                        # additive causal mask applied to the PSUM scores
                        # (cheap DVE PSUM op; keeps the post-exp path clean)
                        o = 2 * p - 4 * j
                        if o >= 0:
                            nc.vector.tensor_add(sps[:], sps[:],
                                                 alw_sb[:, o // 2, :])
                        # pop PE jobs next: their PSUM evictions land ahead
                        # of this pair's exp-gated ops in the engine FIFOs.
                        # bk01 (yps/dps) is idle during pass 1, so jobs may
                        # rotate over it too (deeper eviction pipeline)
                        ops_holder["tags"] = ["bk23", "bk01"]
                        if wo_jobs:
                            wo_jobs.popleft()()
                        if wo_jobs:
                            wo_jobs.popleft()()
                        pt = work.tile([128, 1024], BF, tag="pt", bufs=10,
                                       name="pt")
                        nc.scalar.activation(pt[:], sps[:], AFT.Exp,
                                             bias=bias_sb[:], scale=SCALE)
                        if p == 1:
                            accp = work.tile([128, 1024], F32, tag="accp",
                                             bufs=4, name="accp")
                            eng.tensor_add(accp[:], pts[0][:], pt[:])
                        elif p > 1:
                            eng.tensor_add(accp[:], accp[:], pt[:])
                        pts.append(pt)
                    # pass 2: attn@v accumulation (dense PE). Jobs must NOT
                    # touch bk01 from here on: yps/dps live there and their
                    # readers are emitted after the pops (deadlock otherwise)
                    ops_holder["tags"] = ["bk23"]
                    bkY = bank2("bk01")
                    yps = bkY[:, 0:512]
                    dps = bkY[:, 512:1024]
                    for k in range(K):
                        nc.tensor.matmul(
                            yps, vsb[:, k, :],
                            pts[k // 2][:, 512 * (k % 2):512 * (k % 2) + 512],
                            start=(k == 0), stop=(k == K - 1))
                    # fold pair-accumulator; dense PE jobs cover the latency
                    accb = work.tile([128, 512], BF, tag="accb", bufs=4,
                                     name="accb")
                    eng.tensor_add(accb[:], accp[:, 0:512], accp[:, 512:1024])
                    for _ in range(3):
                        if wo_jobs:
                            wo_jobs.popleft()()
                    nc.tensor.matmul(dps, onesbf_sb[:], accb[:],
                                     start=True, stop=True)
                    rec = work.tile([128, 512], F32, tag="rec", bufs=2,
                                    name="rec")
                    nc.vector.reciprocal_approx_fast(rec[:], dps)
                    yt = work.tile([128, 512], BF, tag="yt", bufs=8,
                                   name="yt")
                    nc.vector.tensor_mul(yt[:], yps, rec[:])
                    yts[h] = yt
                for tl in range(4):
                    for o in range(C // 512):
                        wo_jobs.append(make_wo_job(b, j, tl, o, yts))
            # keep a few jobs alive across the batch boundary so the next
            # batch's first attention chunk has dense PE filler work
            keep = 12 if b < B - 1 else 0
            ops_holder["tags"] = ["bk23", "bk45", "bk67", "bk01"]
            ops_holder["i"] = 0
            while len(wo_jobs) > keep:
                wo_jobs.popleft()()
            ops_holder["tags"] = ["bk23"]
            ops_holder["i"] = 0

    nc.compile()
    return nc


def host_prep(inputs):
    x = np.asarray(inputs["x"], np.float32)
    mask = np.asarray(inputs["mask"], np.float32)
    wq = np.asarray(inputs["wq"], np.float32)
    wk = np.asarray(inputs["wk"], np.float32)
    wv = np.asarray(inputs["wv"], np.float32)
    wo = np.asarray(inputs["wo"], np.float32)

    xT = np.ascontiguousarray(x.reshape(B * T, C).T).astype(bf16)
    inv = 1.0 / (ROPE_BASE ** (np.arange(0, D, 2, dtype=np.float64) / D))
    freqs = np.arange(T, dtype=np.float64)[:, None] * inv[None, :] * B
    emb = np.concatenate([freqs, freqs], axis=-1)       # [T, D]
    cosT = np.cos(emb).T.astype(np.float32).astype(bf16)
    sinT = np.sin(emb).T.astype(np.float32)
    sinT[: D // 2] *= -1.0
    sinTr = sinT.astype(bf16)
    # additive causal mask: -30000 where mask[jj, 128*o + p] says "disallow",
    # 0 elsewhere; stored as two k-tile PAIRS so one DVE add masks a whole
    # [128,1024] PSUM score pair before the exp
    allowA = np.stack([(1.0 - mask[0:512, 128 * o:128 * (o + 1)]).T
                       for o in range(4)], axis=1)            # [128, 4, 512]
    negP = np.ascontiguousarray(
        (allowA.reshape(128, 2, 1024) - 1.0) * 30000.0).astype(bf16)
    ident = np.eye(128, dtype=np.float32).astype(bf16)

    common = dict(xT=xT, cosT=cosT, sinTr=sinTr, negP=negP, ident=ident)
    in_maps = []
    for c in range(NCORES):
        m = dict(common)
        m["wq"] = np.ascontiguousarray(wq[:, 512 * c:512 * (c + 1)]).astype(bf16)
        m["wk"] = np.ascontiguousarray(wk[:, 128 * c:128 * (c + 1)]).astype(bf16)
        m["wv"] = np.ascontiguousarray(wv[:, 128 * c:128 * (c + 1)]).astype(bf16)
        m["woA"] = np.ascontiguousarray(
            wo[512 * c:512 * (c + 1), :].reshape(QH, 128, C)
            .transpose(1, 0, 2)).astype(bf16)
        in_maps.append(m)
    return in_maps


def kernel(**inputs) -> np.ndarray:
    from concourse.bass_utils import run_bass_kernel_spmd

    in_maps = host_prep(inputs)
    nc = emit_program()
    trace = bool(os.environ.get("BASS_KERNEL_TRACE"))
    res = run_bass_kernel_spmd(nc, in_maps, core_ids=list(range(NCORES)),
                               trace=trace)
    if trace and res.exec_time_ns is not None:
        print(f"HW exec time: {res.exec_time_ns} ns")
        if res.instructions_and_trace is not None:
            print("trace:", res.instructions_and_trace[1])
    total = np.zeros((B * T, C), np.float32)
    for r in res.results:
        total += np.asarray(r["out"], np.float32)
    return total.reshape(B, T, C)


# revision 30
# speedup vs baseline: 1.0920x; 1.0077x over previous
"""Trainium2 Bass kernel for GQA attention (B=2, T=2048, C=4096, H=32, KV=8, D=128)
with RoPE and causal mask.

Sharding: tensor-parallel over heads across 8 cores. Each core owns 4 Q heads and
their shared KV head: projects q/k/v for those heads, runs causal attention, and
computes a partial output projection; the host sums the 8 partials (bf16).

All on-chip layouts are transposed ([feature, token]) so every matmul consumes
natural slices:
  qT/kT/vT = W^T @ x  via lhsT=W-tile [128c, cols], rhs=xT-tile [128c, 512t]
  sT[tk, tq] = kT-tile^T @ qT-chunk, two k-tiles paired into one [128,1024]
  PSUM tile so ACT computes exp on both halves in one ACTIVATE
  pT = exp(sT/sqrt(D) - 10); strictly-causal-upper tiles skipped entirely
  yT[d, tq] += v-tile^T @ pT          (v pre-transposed to [t, d] via PE transpose)
  out[tq, :] += yT_h^T @ wo_h         (accumulate 4 heads in PSUM, evict bf16)

Softmax denominator: the exp tiles are pre-summed across key-tiles on DVE
(heads 0,2) / GPSIMD (heads 1,3) into a bf16 row-sum; a single ones-matmul per
(head, chunk) reduces it across partitions. This removes the per-key-tile
ones-matmul stream (~9% of PE cycles) from the tensor engine.

PSUM is managed as ONE kernel-wide pool of four 2-bank [128,1024] tags so no
pool-release barrier ever serializes phase transitions (per-address WAR deps
only): banks 0-1 = pq0/pq1 then yps/dps; banks 2-3 = pq2/pq3 then the wo-job
accumulators; banks 4-5 = pk/pv then odd score pairs; banks 6-7 = V-transpose
ping-pong then even score pairs. At the end of each batch's projections the
V transposes + their evictions are emitted BEFORE the last chunk's RoPE
evictions, so the first attention scores (banks 6-7) start within ~4us of the
last projection matmul and the PE never goes HAM-cold at the phase boundary.
Output-projection matmul "jobs" are popped from a queue inside the score
streams to keep the in-order PE queue dense while ACT works through the exps.
"""

import os
from collections import deque
from contextlib import ExitStack

import numpy as np
import ml_dtypes

import concourse.bacc as bacc
import concourse.mybir as mybir
import concourse.tile as tile

BF = mybir.dt.bfloat16
F32 = mybir.dt.float32
AFT = mybir.ActivationFunctionType

NCORES = 8
B, T, C = 2, 2048, 4096
H, KV, D = 32, 8, 128
QH = H // NCORES          # 4 q-heads per core
CT = C // 128             # 32 contraction tiles
NCH = T // 512            # 4 query chunks per batch
SCALE = 1.0 / float(np.sqrt(D))
EXP_BIAS = -10.0
ROPE_BASE = 10000.0

bf16 = ml_dtypes.bfloat16


def emit_program():
    nc = bacc.Bacc("TRN2", target_bir_lowering=False, debug=False,
                   num_devices=NCORES)

    xT_d = nc.dram_tensor("xT", [C, B * T], BF, kind="ExternalInput").ap()
    wq_d = nc.dram_tensor("wq", [C, QH * D], BF, kind="ExternalInput").ap()
    wk_d = nc.dram_tensor("wk", [C, D], BF, kind="ExternalInput").ap()
    wv_d = nc.dram_tensor("wv", [C, D], BF, kind="ExternalInput").ap()
    wo_d = nc.dram_tensor("woA", [128, QH, C], BF, kind="ExternalInput").ap()
    cos_d = nc.dram_tensor("cosT", [D, T], BF, kind="ExternalInput").ap()
    sin_d = nc.dram_tensor("sinTr", [D, T], BF, kind="ExternalInput").ap()
    alw_d = nc.dram_tensor("negP", [128, 2, 1024], BF, kind="ExternalInput").ap()
    id_d = nc.dram_tensor("ident", [128, 128], BF, kind="ExternalInput").ap()
    out_d = nc.dram_tensor("out", [B * T, C], BF, kind="ExternalOutput").ap()

    with tile.TileContext(nc) as tc, ExitStack() as ctx:
        const = ctx.enter_context(tc.tile_pool(name="const", bufs=1))
        act = ctx.enter_context(tc.tile_pool(name="act", bufs=1))
        work = ctx.enter_context(tc.tile_pool(name="work", bufs=1))
        ps = ctx.enter_context(tc.tile_pool(name="ps", bufs=1, space="PSUM"))

        def bank2(tag):
            return ps.tile([128, 1024], F32, tag=tag, bufs=1, name=tag)

        # weights + tables on the gpsimd DMA queue so they never sit ahead of
        # the xt activation loads (sync queue); chunked in 8-c-tile groups so
        # the first projection matmuls wait on ~1MB, not the full tensors
        wq_sb = const.tile([128, CT, QH * D], BF)
        wk_sb = const.tile([128, CT, D], BF)
        wv_sb = const.tile([128, CT, D], BF)
        xTr = xT_d.rearrange("(ci p) t -> p ci t", p=128)
        wqr = wq_d.rearrange("(ci p) n -> p ci n", p=128)
        wkr = wk_d.rearrange("(ci p) n -> p ci n", p=128)
        wvr = wv_d.rearrange("(ci p) n -> p ci n", p=128)
        GW = 8
        # groups 0-1 up-front on the gpsimd queue; later groups are staged
        # into the sync queue between chunk-0 xt loads so the early xt stream
        # keeps HBM bandwidth (deferred_dma fires inside the ci loop below)
        for g in range(0, 2 * GW, GW):
            s = slice(g, g + GW)
            nc.gpsimd.dma_start(wq_sb[:, s, :], wqr[:, s, :])
            nc.gpsimd.dma_start(wk_sb[:, s, :], wkr[:, s, :])
            nc.gpsimd.dma_start(wv_sb[:, s, :], wvr[:, s, :])
        cos_sb = const.tile([D, T], BF)
        nc.gpsimd.dma_start(cos_sb[:], cos_d)
        sin_sb = const.tile([D, T], BF)
        nc.gpsimd.dma_start(sin_sb[:], sin_d)
        alw_sb = const.tile([128, 2, 1024], BF)
        nc.gpsimd.dma_start(alw_sb[:], alw_d)
        id_sb = const.tile([128, 128], BF)
        nc.gpsimd.dma_start(id_sb[:], id_d)
        wo_sb = const.tile([128, QH, C], BF)

        def stage_weights(g):
            s = slice(g, g + GW)
            nc.sync.dma_start(wq_sb[:, s, :], wqr[:, s, :])
            nc.sync.dma_start(wk_sb[:, s, :], wkr[:, s, :])
            nc.sync.dma_start(wv_sb[:, s, :], wvr[:, s, :])

        deferred_dma = {
            (0, 0, 2): lambda: stage_weights(16),
            (0, 0, 10): lambda: stage_weights(24),
            (0, 1, 8): lambda: nc.sync.dma_start(wo_sb[:], wo_d),
        }
        onesbf_sb = const.tile([128, 128], BF)
        nc.gpsimd.memset(onesbf_sb[:], 1.0)
        bias_sb = const.tile([128, 1], F32)
        nc.gpsimd.memset(bias_sb[:], EXP_BIAS)

        def rope_evict(dst, psum, cs):
            # dst = psum * cos + swap_halves(psum) * sin_rot   (bf16 out)
            # half-swap copies on ACT (fast PSUM reads), muls/add on DVE
            sw = work.tile([128, 512], F32, tag="sw", bufs=3, name="sw")
            nc.scalar.copy(sw[0:64, :], psum[64:128, :])
            nc.scalar.copy(sw[64:128, :], psum[0:64, :])
            nc.vector.tensor_mul(sw[:], sw[:], sin_sb[:, cs])
            cst = work.tile([128, 512], F32, tag="cst", bufs=3, name="cst")
            nc.vector.tensor_mul(cst[:], psum[:], cos_sb[:, cs])
            nc.vector.tensor_add(dst, cst[:], sw[:])

        wo_jobs = deque()
        # wo-job PSUM accumulators ping-pong over half-bank slots; during the
        # end-of-batch drain the idle score banks join the rotation so PE
        # never waits for an eviction
        ops_holder = {"tags": ["bk23"], "i": 0, "tiles": {}}

        def make_wo_job(b, j, tl, o, yts):
            def job():
                tags = ops_holder["tags"]
                slot = ops_holder["i"] % (2 * len(tags))
                tag = tags[slot // 2]
                if slot % 2 == 0:
                    ops_holder["tiles"][tag] = bank2(tag)
                ops = (ops_holder["tiles"][tag][:, 0:512] if slot % 2 == 0
                       else ops_holder["tiles"][tag][:, 512:1024])
                ops_holder["i"] += 1
                for h in range(QH):
                    nc.tensor.matmul(
                        ops, yts[h][:, 128 * tl:128 * (tl + 1)],
                        wo_sb[:, h, 512 * o:512 * (o + 1)],
                        start=h == 0, stop=h == QH - 1)
                ob = work.tile([128, 512], BF, tag="ob", bufs=6,
                               name="ob")
                if (tl + o) % 2 == 0:
                    nc.vector.tensor_copy(ob[:], ops)
                else:
                    nc.scalar.copy(ob[:], ops)
                r0 = b * T + 512 * j + 128 * tl
                nc.sync.dma_start(out_d[r0:r0 + 128, 512 * o:512 * (o + 1)],
                                  ob[:])
            return job

        for b in range(B):
            qT = act.tile([D, QH, T], BF, tag="qT", name="qT")
            kT = act.tile([D, T], BF, tag="kT", name="kT")
            vT = act.tile([D, T], BF, tag="vT", name="vT")
            vsb = act.tile([128, T // 128, D], BF, tag="v", name="vsb")

            # ---- projections ----
            # banks 0-1 = pq0,pq1; banks 2-3 = pq2,pq3; banks 4-5 = pk,pv
            for jc in range(NCH):
                bkA = bank2("bk01")
                bkB = bank2("bk23")
                bkC = bank2("bk45")
                pq = [bkA[:, 0:512], bkA[:, 512:1024],
                      bkB[:, 0:512], bkB[:, 512:1024]]
                pk = bkC[:, 0:512]
                pv = bkC[:, 512:1024]
                # q matmuls run SKEW c-tiles behind k/v so the previous
                # chunk's pq bank evictions are hidden behind ready work
                SKEW = 4
                xts = {}
                col0 = b * T + 512 * jc

                def q_mms(cq):
                    for h in range(QH):
                        nc.tensor.matmul(
                            pq[h], wq_sb[:, cq, 128 * h:128 * (h + 1)],
                            xts[cq], start=cq == 0, stop=cq == CT - 1)
                    if cq >= SKEW:
                        del xts[cq - SKEW]

                for ci in range(CT):
                    if (b, jc, ci) in deferred_dma:
                        deferred_dma.pop((b, jc, ci))()
                    if ci % 2 == 0:
                        # one DMA covers two contraction tiles (fewer, larger
                        # transfers keep the activation stream ahead of PE)
                        xt2 = work.tile([128, 2, 512], BF, tag="xt", bufs=6,
                                        name="xt2")
                        nc.sync.dma_start(
                            xt2[:], xTr[:, ci:ci + 2, col0:col0 + 512])
                        xts[ci] = xt2[:, 0, :]
                        xts[ci + 1] = xt2[:, 1, :]
                    st, sp = ci == 0, ci == CT - 1
                    nc.tensor.matmul(pk, wk_sb[:, ci, :], xts[ci],
                                     start=st, stop=sp)
                    nc.tensor.matmul(pv, wv_sb[:, ci, :], xts[ci],
                                     start=st, stop=sp)
                    if ci >= SKEW:
                        q_mms(ci - SKEW)
                for cq in range(CT - SKEW, CT):
                    q_mms(cq)
                cs = slice(512 * jc, 512 * (jc + 1))
                nc.scalar.copy(vT[:, cs], pv)
                last = jc == NCH - 1
                if last:
                    # transposes first (banks 6-7, free now) so attention can
                    # start while the last chunk's RoPE evictions trail
                    bkDv = bank2("bk67").bitcast(BF)
                    for k in range(T // 128):
                        tp = (bkDv[:, 0:128] if k % 2 == 0
                              else bkDv[:, 1024:1152])
                        nc.tensor.transpose(tp, vT[:, 128 * k:128 * (k + 1)],
                                            id_sb[:])
                        if k % 2 == 0:
                            nc.vector.tensor_copy(vsb[:, k, :], tp)
                        else:
                            nc.scalar.copy(vsb[:, k, :], tp)
                    rope_evict(qT[:, 0, cs], pq[0], cs)
                    rope_evict(qT[:, 1, cs], pq[1], cs)
                    rope_evict(kT[:, cs], pk, cs)
                    rope_evict(qT[:, 2, cs], pq[2], cs)
                    rope_evict(qT[:, 3, cs], pq[3], cs)
                else:
                    rope_evict(kT[:, cs], pk, cs)
                    for h in range(QH):
                        rope_evict(qT[:, h, cs], pq[h], cs)

            # ---- attention + output projection ----
            # banks 0-1 = yps,dps; banks 2-3 = wo accumulators; 4-7 = scores
            for j in range(NCH):
                yts = {}
                K = 4 * j + 4
                P = K // 2
                for h in range(QH):
                    # denominator pre-sum engine: DVE for the first/last head
                    # (short latency to the ones-matmul), GPSIMD for the rest
                    eng = nc.vector if h in (0, QH - 1) else nc.gpsimd
                    qs = qT[:, h, 512 * j:512 * (j + 1)]
                    # pass 1: paired score matmuls stream; paired exp trails
                    # on ACT; pair-level key pre-sum trails on DVE/GPSIMD
                    pts = []
                    accp = None
                    npop = 2 if j in (0, NCH - 1) else 1
                    for p in range(P):
                        sps = bank2("bk67") if p % 2 == 0 else bank2("bk45")
                        # skip score columns that the causal mask fully
                        # zeroes anyway (the -30000 additive mask covers the
                        # stale PSUM there); k-tile 4j+ot masks cols < 128*ot
                        for half in range(2):
                            kt = 2 * p + half
                            skip = max(0, 128 * (kt - 4 * j))
                            nc.tensor.matmul(
                                sps[:, 512 * half + skip:512 * (half + 1)],
                                kT[:, 128 * kt:128 * (kt + 1)],
                                qs[:, skip:512], start=True, stop=True)
                        # additive causal mask applied to the PSUM scores
                        # (cheap DVE PSUM op; keeps the post-exp path clean)
                        o = 2 * p - 4 * j
                        if o >= 0:
                            nc.vector.tensor_add(sps[:], sps[:],
                                                 alw_sb[:, o // 2, :])
                        # pop PE jobs next: their PSUM evictions land ahead
                        # of this pair's exp-gated ops in the engine FIFOs.
                        # bk01 (yps/dps) is idle during pass 1, so jobs may
                        # rotate over it too (deeper eviction pipeline);
                        # pop rate per chunk spreads the 32 jobs evenly
                        ops_holder["tags"] = ["bk23", "bk01"]
                        for _ in range(npop):
                            if wo_jobs:
                                wo_jobs.popleft()()
                        pt = work.tile([128, 1024], BF, tag="pt", bufs=10,
                                       name="pt")
                        nc.scalar.activation(pt[:], sps[:], AFT.Exp,
                                             bias=bias_sb[:], scale=SCALE)
                        if p == 1:
                            accp = work.tile([128, 1024], F32, tag="accp",
                                             bufs=4, name="accp")
                            eng.tensor_add(accp[:], pts[0][:], pt[:])
                        elif p > 1:
                            eng.tensor_add(accp[:], accp[:], pt[:])
                        pts.append(pt)
                    # pass 2: attn@v accumulation (dense PE). Jobs must NOT
                    # touch bk01 from here on: yps/dps live there and their
                    # readers are emitted after the pops (deadlock otherwise)
                    ops_holder["tags"] = ["bk23"]
                    bkY = bank2("bk01")
                    yps = bkY[:, 0:512]
                    dps = bkY[:, 512:1024]
                    for k in range(K):
                        nc.tensor.matmul(
                            yps, vsb[:, k, :],
                            pts[k // 2][:, 512 * (k % 2):512 * (k % 2) + 512],
                            start=(k == 0), stop=(k == K - 1))
                    # fold pair-accumulator; dense PE jobs cover the latency
                    accb = work.tile([128, 512], BF, tag="accb", bufs=4,
                                     name="accb")
                    eng.tensor_add(accb[:], accp[:, 0:512], accp[:, 512:1024])
                    for _ in range(3):
                        if wo_jobs:
                            wo_jobs.popleft()()
                    nc.tensor.matmul(dps, onesbf_sb[:], accb[:],
                                     start=True, stop=True)
                    rec = work.tile([128, 512], F32, tag="rec", bufs=2,
                                    name="rec")
                    nc.vector.reciprocal_approx_fast(rec[:], dps)
                    yt = work.tile([128, 512], BF, tag="yt", bufs=8,
                                   name="yt")
                    nc.vector.tensor_mul(yt[:], yps, rec[:])
                    yts[h] = yt
                for tl in range(4):
                    for o in range(C // 512):
                        wo_jobs.append(make_wo_job(b, j, tl, o, yts))
            # keep a few jobs alive across the batch boundary so the next
            # batch's first attention chunk has dense PE filler work
            keep = 12 if b < B - 1 else 0
            ops_holder["tags"] = ["bk23", "bk45", "bk67", "bk01"]
            ops_holder["i"] = 0
            while len(wo_jobs) > keep:
                wo_jobs.popleft()()
            ops_holder["tags"] = ["bk23"]
            ops_holder["i"] = 0

    nc.compile()
    return nc


def host_prep(inputs):
    x = np.asarray(inputs["x"], np.float32)
    mask = np.asarray(inputs["mask"], np.float32)
    wq = np.asarray(inputs["wq"], np.float32)
    wk = np.asarray(inputs["wk"], np.float32)
    wv = np.asarray(inputs["wv"], np.float32)
    wo = np.asarray(inputs["wo"], np.float32)

    xT = np.ascontiguousarray(x.reshape(B * T, C).T).astype(bf16)
    inv = 1.0 / (ROPE_BASE ** (np.arange(0, D, 2, dtype=np.float64) / D))
    freqs = np.arange(T, dtype=np.float64)[:, None] * inv[None, :] * B
    emb = np.concatenate([freqs, freqs], axis=-1)       # [T, D]
    cosT = np.cos(emb).T.astype(np.float32).astype(bf16)
    sinT = np.sin(emb).T.astype(np.float32)
    sinT[: D // 2] *= -1.0
    sinTr = sinT.astype(bf16)
    # additive causal mask: -30000 where mask[jj, 128*o + p] says "disallow",
    # 0 elsewhere; stored as two k-tile PAIRS so one DVE add masks a whole
    # [128,1024] PSUM score pair before the exp
    allowA = np.stack([(1.0 - mask[0:512, 128 * o:128 * (o + 1)]).T
                       for o in range(4)], axis=1)            # [128, 4, 512]
    negP = np.ascontiguousarray(
        (allowA.reshape(128, 2, 1024) - 1.0) * 30000.0).astype(bf16)
    ident = np.eye(128, dtype=np.float32).astype(bf16)

    common = dict(xT=xT, cosT=cosT, sinTr=sinTr, negP=negP, ident=ident)
    in_maps = []
    for c in range(NCORES):
        m = dict(common)
        m["wq"] = np.ascontiguousarray(wq[:, 512 * c:512 * (c + 1)]).astype(bf16)
        m["wk"] = np.ascontiguousarray(wk[:, 128 * c:128 * (c + 1)]).astype(bf16)
        m["wv"] = np.ascontiguousarray(wv[:, 128 * c:128 * (c + 1)]).astype(bf16)
        m["woA"] = np.ascontiguousarray(
            wo[512 * c:512 * (c + 1), :].reshape(QH, 128, C)
            .transpose(1, 0, 2)).astype(bf16)
        in_maps.append(m)
    return in_maps


def kernel(**inputs) -> np.ndarray:
    from concourse.bass_utils import run_bass_kernel_spmd

    in_maps = host_prep(inputs)
    nc = emit_program()
    trace = bool(os.environ.get("BASS_KERNEL_TRACE"))
    res = run_bass_kernel_spmd(nc, in_maps, core_ids=list(range(NCORES)),
                               trace=trace)
    if trace and res.exec_time_ns is not None:
        print(f"HW exec time: {res.exec_time_ns} ns")
        if res.instructions_and_trace is not None:
            print("trace:", res.instructions_and_trace[1])
    total = np.zeros((B * T, C), np.float32)
    for r in res.results:
        total += np.asarray(r["out"], np.float32)
    return total.reshape(B, T, C)


# revision 32
# speedup vs baseline: 1.1144x; 1.0205x over previous
"""Trainium2 Bass kernel for GQA attention (B=2, T=2048, C=4096, H=32, KV=8, D=128)
with RoPE and causal mask.

Sharding: tensor-parallel over heads across 8 cores. Each core owns 4 Q heads and
their shared KV head: projects q/k/v for those heads, runs causal attention, and
computes a partial output projection; the host sums the 8 partials (bf16).

All on-chip layouts are transposed ([feature, token]) so every matmul consumes
natural slices:
  qT/kT/vT = W^T @ x  via lhsT=W-tile [128c, cols], rhs=xT-tile [128c, 512t]
  sT[tk, tq] = kT-tile^T @ qT-chunk, two k-tiles paired into one [128,1024]
  PSUM tile so ACT computes exp on both halves in one ACTIVATE
  pT = exp(sT/sqrt(D) - 10); strictly-causal-upper tiles skipped entirely
  yT[d, tq] += v-tile^T @ pT          (v pre-transposed to [t, d] via PE transpose)
  out[tq, :] += yT_h^T @ wo_h         (accumulate 4 heads in PSUM, evict bf16)

Softmax denominator: the exp tiles are pre-summed across key-tiles on DVE
(heads 0,2) / GPSIMD (heads 1,3) into a bf16 row-sum; a single ones-matmul per
(head, chunk) reduces it across partitions. This removes the per-key-tile
ones-matmul stream (~9% of PE cycles) from the tensor engine.

PSUM is managed as ONE kernel-wide pool of four 2-bank [128,1024] tags so no
pool-release barrier ever serializes phase transitions (per-address WAR deps
only): banks 0-1 = pq0/pq1 then yps/dps; banks 2-3 = pq2/pq3 then the wo-job
accumulators; banks 4-5 = pk/pv then odd score pairs; banks 6-7 = V-transpose
ping-pong then even score pairs. At the end of each batch's projections the
V transposes + their evictions are emitted BEFORE the last chunk's RoPE
evictions, so the first attention scores (banks 6-7) start within ~4us of the
last projection matmul and the PE never goes HAM-cold at the phase boundary.
Output-projection matmul "jobs" are popped from a queue inside the score
streams to keep the in-order PE queue dense while ACT works through the exps.
"""

import os
from collections import deque
from contextlib import ExitStack

import numpy as np
import ml_dtypes

import concourse.bacc as bacc
import concourse.mybir as mybir
import concourse.tile as tile

BF = mybir.dt.bfloat16
F32 = mybir.dt.float32
AFT = mybir.ActivationFunctionType

NCORES = 8
B, T, C = 2, 2048, 4096
H, KV, D = 32, 8, 128
QH = H // NCORES          # 4 q-heads per core
CT = C // 128             # 32 contraction tiles
NCH = T // 512            # 4 query chunks per batch
SCALE = 1.0 / float(np.sqrt(D))
EXP_BIAS = -10.0
ROPE_BASE = 10000.0

bf16 = ml_dtypes.bfloat16


def emit_program():
    nc = bacc.Bacc("TRN2", target_bir_lowering=False, debug=False,
                   num_devices=NCORES)

    xT_d = nc.dram_tensor("xT", [C, B * T], BF, kind="ExternalInput").ap()
    wq_d = nc.dram_tensor("wq", [C, QH * D], BF, kind="ExternalInput").ap()
    wk_d = nc.dram_tensor("wk", [C, D], BF, kind="ExternalInput").ap()
    wv_d = nc.dram_tensor("wv", [C, D], BF, kind="ExternalInput").ap()
    wo_d = nc.dram_tensor("woA", [128, QH, C], BF, kind="ExternalInput").ap()
    cos_d = nc.dram_tensor("cosT", [D, T], BF, kind="ExternalInput").ap()
    sin_d = nc.dram_tensor("sinTr", [D, T], BF, kind="ExternalInput").ap()
    alw_d = nc.dram_tensor("negP", [128, 2, 1024], BF, kind="ExternalInput").ap()
    id_d = nc.dram_tensor("ident", [128, 128], BF, kind="ExternalInput").ap()
    out_d = nc.dram_tensor("out", [B * T, C], BF, kind="ExternalOutput").ap()

    with tile.TileContext(nc) as tc, ExitStack() as ctx:
        const = ctx.enter_context(tc.tile_pool(name="const", bufs=1))
        act = ctx.enter_context(tc.tile_pool(name="act", bufs=1))
        work = ctx.enter_context(tc.tile_pool(name="work", bufs=1))
        ps = ctx.enter_context(tc.tile_pool(name="ps", bufs=1, space="PSUM"))

        def bank2(tag):
            return ps.tile([128, 1024], F32, tag=tag, bufs=1, name=tag)

        # weights + tables on the gpsimd DMA queue so they never sit ahead of
        # the xt activation loads (sync queue); chunked in 8-c-tile groups so
        # the first projection matmuls wait on ~1MB, not the full tensors
        wq_sb = const.tile([128, CT, QH * D], BF)
        wk_sb = const.tile([128, CT, D], BF)
        wv_sb = const.tile([128, CT, D], BF)
        xTr = xT_d.rearrange("(ci p) t -> p ci t", p=128)
        wqr = wq_d.rearrange("(ci p) n -> p ci n", p=128)
        wkr = wk_d.rearrange("(ci p) n -> p ci n", p=128)
        wvr = wv_d.rearrange("(ci p) n -> p ci n", p=128)
        GW = 8
        # groups 0-1 up-front on the gpsimd queue; later groups are staged
        # into the sync queue between chunk-0 xt loads so the early xt stream
        # keeps HBM bandwidth (deferred_dma fires inside the ci loop below)
        for g in range(0, 2 * GW, GW):
            s = slice(g, g + GW)
            nc.gpsimd.dma_start(wq_sb[:, s, :], wqr[:, s, :])
            nc.gpsimd.dma_start(wk_sb[:, s, :], wkr[:, s, :])
            nc.gpsimd.dma_start(wv_sb[:, s, :], wvr[:, s, :])
        cos_sb = const.tile([D, T], BF)
        nc.gpsimd.dma_start(cos_sb[:], cos_d)
        sin_sb = const.tile([D, T], BF)
        nc.gpsimd.dma_start(sin_sb[:], sin_d)
        alw_sb = const.tile([128, 2, 1024], BF)
        nc.gpsimd.dma_start(alw_sb[:], alw_d)
        id_sb = const.tile([128, 128], BF)
        nc.gpsimd.dma_start(id_sb[:], id_d)
        wo_sb = const.tile([128, QH, C], BF)

        def stage_weights(g):
            s = slice(g, g + GW)
            nc.sync.dma_start(wq_sb[:, s, :], wqr[:, s, :])
            nc.sync.dma_start(wk_sb[:, s, :], wkr[:, s, :])
            nc.sync.dma_start(wv_sb[:, s, :], wvr[:, s, :])

        deferred_dma = {
            (0, 0, 2): lambda: stage_weights(16),
            (0, 0, 10): lambda: stage_weights(24),
            (0, 1, 8): lambda: nc.sync.dma_start(wo_sb[:], wo_d),
        }
        onesbf_sb = const.tile([128, 128], BF)
        nc.gpsimd.memset(onesbf_sb[:], 1.0)
        bias_sb = const.tile([128, 1], F32)
        nc.gpsimd.memset(bias_sb[:], EXP_BIAS)

        def rope_evict(dst, psum, cs):
            # dst = psum * cos + swap_halves(psum) * sin_rot   (bf16 out)
            # half-swap copies on ACT (fast PSUM reads), muls/add on DVE
            sw = work.tile([128, 512], F32, tag="sw", bufs=3, name="sw")
            nc.scalar.copy(sw[0:64, :], psum[64:128, :])
            nc.scalar.copy(sw[64:128, :], psum[0:64, :])
            nc.vector.tensor_mul(sw[:], sw[:], sin_sb[:, cs])
            cst = work.tile([128, 512], F32, tag="cst", bufs=3, name="cst")
            nc.vector.tensor_mul(cst[:], psum[:], cos_sb[:, cs])
            nc.vector.tensor_add(dst, cst[:], sw[:])

        wo_jobs = deque()
        # wo-job PSUM accumulators ping-pong over half-bank slots; during the
        # end-of-batch drain the idle score banks join the rotation so PE
        # never waits for an eviction
        ops_holder = {"tags": ["bk23"], "i": 0, "tiles": {}}

        def make_wo_job(b, j, tl, o, yts):
            def job():
                tags = ops_holder["tags"]
                slot = ops_holder["i"] % (2 * len(tags))
                tag = tags[slot // 2]
                if slot % 2 == 0:
                    ops_holder["tiles"][tag] = bank2(tag)
                ops = (ops_holder["tiles"][tag][:, 0:512] if slot % 2 == 0
                       else ops_holder["tiles"][tag][:, 512:1024])
                ops_holder["i"] += 1
                for h in range(QH):
                    nc.tensor.matmul(
                        ops, yts[h][:, 128 * tl:128 * (tl + 1)],
                        wo_sb[:, h, 512 * o:512 * (o + 1)],
                        start=h == 0, stop=h == QH - 1)
                ob = work.tile([128, 512], BF, tag="ob", bufs=6,
                               name="ob")
                if (tl + o) % 2 == 0:
                    nc.vector.tensor_copy(ob[:], ops)
                else:
                    nc.scalar.copy(ob[:], ops)
                r0 = b * T + 512 * j + 128 * tl
                nc.sync.dma_start(out_d[r0:r0 + 128, 512 * o:512 * (o + 1)],
                                  ob[:])
            return job

        for b in range(B):
            qT = act.tile([D, QH, T], BF, tag="qT", name="qT")
            kT = act.tile([D, T], BF, tag="kT", name="kT")
            vT = act.tile([D, T], BF, tag="vT", name="vT")
            vsb = act.tile([128, T // 128, D], BF, tag="v", name="vsb")

            # ---- projections ----
            # banks 0-1 = pq0,pq1; banks 2-3 = pq2,pq3; banks 4-5 = pk,pv
            for jc in range(NCH):
                bkA = bank2("bk01")
                bkB = bank2("bk23")
                bkC = bank2("bk45")
                pq = [bkA[:, 0:512], bkA[:, 512:1024],
                      bkB[:, 0:512], bkB[:, 512:1024]]
                pk = bkC[:, 0:512]
                pv = bkC[:, 512:1024]
                # q matmuls run SKEW c-tiles behind k/v so the previous
                # chunk's pq bank evictions are hidden behind ready work
                SKEW = 4
                xts = {}
                col0 = b * T + 512 * jc

                def q_mms(cq):
                    for h in range(QH):
                        nc.tensor.matmul(
                            pq[h], wq_sb[:, cq, 128 * h:128 * (h + 1)],
                            xts[cq], start=cq == 0, stop=cq == CT - 1)
                    if cq >= SKEW:
                        del xts[cq - SKEW]

                for ci in range(CT):
                    if (b, jc, ci) in deferred_dma:
                        deferred_dma.pop((b, jc, ci))()
                    if ci % 2 == 0:
                        # one DMA covers two contraction tiles (fewer, larger
                        # transfers keep the activation stream ahead of PE)
                        xt2 = work.tile([128, 2, 512], BF, tag="xt", bufs=6,
                                        name="xt2")
                        nc.sync.dma_start(
                            xt2[:], xTr[:, ci:ci + 2, col0:col0 + 512])
                        xts[ci] = xt2[:, 0, :]
                        xts[ci + 1] = xt2[:, 1, :]
                    st, sp = ci == 0, ci == CT - 1
                    nc.tensor.matmul(pk, wk_sb[:, ci, :], xts[ci],
                                     start=st, stop=sp)
                    nc.tensor.matmul(pv, wv_sb[:, ci, :], xts[ci],
                                     start=st, stop=sp)
                    if ci >= SKEW:
                        q_mms(ci - SKEW)
                for cq in range(CT - SKEW, CT):
                    q_mms(cq)
                cs = slice(512 * jc, 512 * (jc + 1))
                nc.scalar.copy(vT[:, cs], pv)
                last = jc == NCH - 1
                if last:
                    # transposes first (banks 6-7, free now) so attention can
                    # start while the last chunk's RoPE evictions trail
                    bkDv = bank2("bk67").bitcast(BF)
                    for k in range(T // 128):
                        tp = (bkDv[:, 0:128] if k % 2 == 0
                              else bkDv[:, 1024:1152])
                        nc.tensor.transpose(tp, vT[:, 128 * k:128 * (k + 1)],
                                            id_sb[:])
                        if k % 2 == 0:
                            nc.vector.tensor_copy(vsb[:, k, :], tp)
                        else:
                            nc.scalar.copy(vsb[:, k, :], tp)
                    rope_evict(qT[:, 0, cs], pq[0], cs)
                    rope_evict(qT[:, 1, cs], pq[1], cs)
                    rope_evict(kT[:, cs], pk, cs)
                    rope_evict(qT[:, 2, cs], pq[2], cs)
                    rope_evict(qT[:, 3, cs], pq[3], cs)
                else:
                    rope_evict(kT[:, cs], pk, cs)
                    for h in range(QH):
                        rope_evict(qT[:, h, cs], pq[h], cs)

            # ---- attention + output projection ----
            # banks 0-1 = yps,dps; banks 2-3 = wo accumulators; 4-7 = scores
            for j in range(NCH):
                yts = {}
                K = 4 * j + 4
                P = K // 2
                for h in range(QH):
                    # denominator pre-sum engine: DVE for the first/last head
                    # (short latency to the ones-matmul), GPSIMD for the rest
                    eng = nc.vector if h in (0, QH - 1) else nc.gpsimd
                    qs = qT[:, h, 512 * j:512 * (j + 1)]
                    # pass 1: paired score matmuls stream; paired exp trails
                    # on ACT; pair-level key pre-sum trails on DVE/GPSIMD
                    pts = []
                    accp = None
                    npop = 2 if j in (0, NCH - 1) else 1
                    for p in range(P):
                        sps = bank2("bk67") if p % 2 == 0 else bank2("bk45")
                        # skip score columns that the causal mask fully
                        # zeroes anyway (the -30000 additive mask covers the
                        # stale PSUM there); k-tile 4j+ot masks cols < 128*ot
                        for half in range(2):
                            kt = 2 * p + half
                            skip = max(0, 128 * (kt - 4 * j))
                            nc.tensor.matmul(
                                sps[:, 512 * half + skip:512 * (half + 1)],
                                kT[:, 128 * kt:128 * (kt + 1)],
                                qs[:, skip:512], start=True, stop=True)
                        # additive causal mask applied to the PSUM scores
                        # (cheap DVE PSUM op; keeps the post-exp path clean)
                        o = 2 * p - 4 * j
                        if o >= 0:
                            nc.vector.tensor_add(sps[:], sps[:],
                                                 alw_sb[:, o // 2, :])
                        # pop PE jobs next: their PSUM evictions land ahead
                        # of this pair's exp-gated ops in the engine FIFOs.
                        # bk01 (yps/dps) is idle during pass 1, so jobs may
                        # rotate over it too (deeper eviction pipeline);
                        # pop rate per chunk spreads the 32 jobs evenly
                        ops_holder["tags"] = ["bk23", "bk01"]
                        for _ in range(npop):
                            if wo_jobs:
                                wo_jobs.popleft()()
                        pt = work.tile([128, 1024], BF, tag="pt", bufs=10,
                                       name="pt")
                        nc.scalar.activation(pt[:], sps[:], AFT.Exp,
                                             bias=bias_sb[:], scale=SCALE)
                        # bf16 ping-pong pair-accumulator (out-of-place: DVE
                        # 2x packing; final add always on DVE for low latency
                        # to the denominator matmuls)
                        e = nc.vector if p == P - 1 else eng
                        if p == 1:
                            accp = work.tile([128, 1024], BF, tag="accp",
                                             bufs=8, name="accp")
                            e.tensor_add(accp[:], pts[0][:], pt[:])
                        elif p > 1:
                            accp2 = work.tile([128, 1024], BF, tag="accp",
                                              bufs=8, name="accp")
                            e.tensor_add(accp2[:], accp[:], pt[:])
                            accp = accp2
                        pts.append(pt)
                    # pass 2: attn@v accumulation (dense PE). Jobs must NOT
                    # touch bk01 from here on: yps/dps live there and their
                    # readers are emitted after the pops (deadlock otherwise)
                    ops_holder["tags"] = ["bk23"]
                    bkY = bank2("bk01")
                    yps = bkY[:, 0:512]
                    dps = bkY[:, 512:1024]
                    for k in range(K):
                        nc.tensor.matmul(
                            yps, vsb[:, k, :],
                            pts[k // 2][:, 512 * (k % 2):512 * (k % 2) + 512],
                            start=(k == 0), stop=(k == K - 1))
                    for _ in range(3):
                        if wo_jobs:
                            wo_jobs.popleft()()
                    # denominator: two accumulating ones-matmuls over the
                    # bf16 pair-accumulator halves (no fold needed)
                    nc.tensor.matmul(dps, onesbf_sb[:], accp[:, 0:512],
                                     start=True, stop=False)
                    nc.tensor.matmul(dps, onesbf_sb[:], accp[:, 512:1024],
                                     start=False, stop=True)
                    rec = work.tile([128, 512], F32, tag="rec", bufs=2,
                                    name="rec")
                    nc.vector.reciprocal_approx_fast(rec[:], dps)
                    yt = work.tile([128, 512], BF, tag="yt", bufs=8,
                                   name="yt")
                    nc.vector.tensor_mul(yt[:], yps, rec[:])
                    yts[h] = yt
                for tl in range(4):
                    for o in range(C // 512):
                        wo_jobs.append(make_wo_job(b, j, tl, o, yts))
            # keep a few jobs alive across the batch boundary so the next
            # batch's first attention chunk has dense PE filler work
            keep = 12 if b < B - 1 else 0
            ops_holder["tags"] = ["bk23", "bk45", "bk67", "bk01"]
            ops_holder["i"] = 0
            while len(wo_jobs) > keep:
                wo_jobs.popleft()()
            ops_holder["tags"] = ["bk23"]
            ops_holder["i"] = 0

    nc.compile()
    return nc


def host_prep(inputs):
    x = np.asarray(inputs["x"], np.float32)
    mask = np.asarray(inputs["mask"], np.float32)
    wq = np.asarray(inputs["wq"], np.float32)
    wk = np.asarray(inputs["wk"], np.float32)
    wv = np.asarray(inputs["wv"], np.float32)
    wo = np.asarray(inputs["wo"], np.float32)

    xT = np.ascontiguousarray(x.reshape(B * T, C).T).astype(bf16)
    inv = 1.0 / (ROPE_BASE ** (np.arange(0, D, 2, dtype=np.float64) / D))
    freqs = np.arange(T, dtype=np.float64)[:, None] * inv[None, :] * B
    emb = np.concatenate([freqs, freqs], axis=-1)       # [T, D]
    cosT = np.cos(emb).T.astype(np.float32).astype(bf16)
    sinT = np.sin(emb).T.astype(np.float32)
    sinT[: D // 2] *= -1.0
    sinTr = sinT.astype(bf16)
    # additive causal mask: -30000 where mask[jj, 128*o + p] says "disallow",
    # 0 elsewhere; stored as two k-tile PAIRS so one DVE add masks a whole
    # [128,1024] PSUM score pair before the exp
    allowA = np.stack([(1.0 - mask[0:512, 128 * o:128 * (o + 1)]).T
                       for o in range(4)], axis=1)            # [128, 4, 512]
    negP = np.ascontiguousarray(
        (allowA.reshape(128, 2, 1024) - 1.0) * 30000.0).astype(bf16)
    ident = np.eye(128, dtype=np.float32).astype(bf16)

    common = dict(xT=xT, cosT=cosT, sinTr=sinTr, negP=negP, ident=ident)
    in_maps = []
    for c in range(NCORES):
        m = dict(common)
        m["wq"] = np.ascontiguousarray(wq[:, 512 * c:512 * (c + 1)]).astype(bf16)
        m["wk"] = np.ascontiguousarray(wk[:, 128 * c:128 * (c + 1)]).astype(bf16)
        m["wv"] = np.ascontiguousarray(wv[:, 128 * c:128 * (c + 1)]).astype(bf16)
        m["woA"] = np.ascontiguousarray(
            wo[512 * c:512 * (c + 1), :].reshape(QH, 128, C)
            .transpose(1, 0, 2)).astype(bf16)
        in_maps.append(m)
    return in_maps


def kernel(**inputs) -> np.ndarray:
    from concourse.bass_utils import run_bass_kernel_spmd

    in_maps = host_prep(inputs)
    nc = emit_program()
    trace = bool(os.environ.get("BASS_KERNEL_TRACE"))
    res = run_bass_kernel_spmd(nc, in_maps, core_ids=list(range(NCORES)),
                               trace=trace)
    if trace and res.exec_time_ns is not None:
        print(f"HW exec time: {res.exec_time_ns} ns")
        if res.instructions_and_trace is not None:
            print("trace:", res.instructions_and_trace[1])
    total = np.zeros((B * T, C), np.float32)
    for r in res.results:
        total += np.asarray(r["out"], np.float32)
    return total.reshape(B, T, C)


# revision 33
# speedup vs baseline: 1.1424x; 1.0251x over previous
"""Trainium2 Bass kernel for GQA attention (B=2, T=2048, C=4096, H=32, KV=8, D=128)
with RoPE and causal mask.

Sharding: tensor-parallel over heads across 8 cores. Each core owns 4 Q heads and
their shared KV head: projects q/k/v for those heads, runs causal attention, and
computes a partial output projection; the host sums the 8 partials (bf16).

All on-chip layouts are transposed ([feature, token]) so every matmul consumes
natural slices:
  qT/kT/vT = W^T @ x  via lhsT=W-tile [128c, cols], rhs=xT-tile [128c, 512t]
  sT[tk, tq] = kT-tile^T @ qT-chunk, two k-tiles paired into one [128,1024]
  PSUM tile so ACT computes exp on both halves in one ACTIVATE
  pT = exp(sT/sqrt(D) - 10); strictly-causal-upper tiles skipped entirely
  yT[d, tq] += v-tile^T @ pT          (v pre-transposed to [t, d] via PE transpose)
  out[tq, :] += yT_h^T @ wo_h         (accumulate 4 heads in PSUM, evict bf16)

Softmax denominator: the exp tiles are pre-summed across key-tiles on DVE
(heads 0,2) / GPSIMD (heads 1,3) into a bf16 row-sum; a single ones-matmul per
(head, chunk) reduces it across partitions. This removes the per-key-tile
ones-matmul stream (~9% of PE cycles) from the tensor engine.

PSUM is managed as ONE kernel-wide pool of four 2-bank [128,1024] tags so no
pool-release barrier ever serializes phase transitions (per-address WAR deps
only): banks 0-1 = pq0/pq1 then yps/dps; banks 2-3 = pq2/pq3 then the wo-job
accumulators; banks 4-5 = pk/pv then odd score pairs; banks 6-7 = V-transpose
ping-pong then even score pairs. At the end of each batch's projections the
V transposes + their evictions are emitted BEFORE the last chunk's RoPE
evictions, so the first attention scores (banks 6-7) start within ~4us of the
last projection matmul and the PE never goes HAM-cold at the phase boundary.
Output-projection matmul "jobs" are popped from a queue inside the score
streams to keep the in-order PE queue dense while ACT works through the exps.
"""

import os
from collections import deque
from contextlib import ExitStack

import numpy as np
import ml_dtypes

import concourse.bacc as bacc
import concourse.mybir as mybir
import concourse.tile as tile

BF = mybir.dt.bfloat16
F32 = mybir.dt.float32
AFT = mybir.ActivationFunctionType

NCORES = 8
B, T, C = 2, 2048, 4096
H, KV, D = 32, 8, 128
QH = H // NCORES          # 4 q-heads per core
CT = C // 128             # 32 contraction tiles
NCH = T // 512            # 4 query chunks per batch
SCALE = 1.0 / float(np.sqrt(D))
EXP_BIAS = -10.0
ROPE_BASE = 10000.0

bf16 = ml_dtypes.bfloat16


def emit_program():
    nc = bacc.Bacc("TRN2", target_bir_lowering=False, debug=False,
                   num_devices=NCORES)

    xT_d = nc.dram_tensor("xT", [C, B * T], BF, kind="ExternalInput").ap()
    wq_d = nc.dram_tensor("wq", [C, QH * D], BF, kind="ExternalInput").ap()
    wk_d = nc.dram_tensor("wk", [C, D], BF, kind="ExternalInput").ap()
    wv_d = nc.dram_tensor("wv", [C, D], BF, kind="ExternalInput").ap()
    wo_d = nc.dram_tensor("woA", [128, QH, C], BF, kind="ExternalInput").ap()
    cos_d = nc.dram_tensor("cosT", [D, T], BF, kind="ExternalInput").ap()
    sin_d = nc.dram_tensor("sinTr", [D, T], BF, kind="ExternalInput").ap()
    alw_d = nc.dram_tensor("negP", [128, 2, 1024], BF, kind="ExternalInput").ap()
    id_d = nc.dram_tensor("ident", [128, 128], BF, kind="ExternalInput").ap()
    out_d = nc.dram_tensor("out", [B * T, C], BF, kind="ExternalOutput").ap()

    with tile.TileContext(nc) as tc, ExitStack() as ctx:
        const = ctx.enter_context(tc.tile_pool(name="const", bufs=1))
        act = ctx.enter_context(tc.tile_pool(name="act", bufs=1))
        work = ctx.enter_context(tc.tile_pool(name="work", bufs=1))
        ps = ctx.enter_context(tc.tile_pool(name="ps", bufs=1, space="PSUM"))

        def bank2(tag):
            return ps.tile([128, 1024], F32, tag=tag, bufs=1, name=tag)

        # weights + tables on the gpsimd DMA queue so they never sit ahead of
        # the xt activation loads (sync queue); chunked in 8-c-tile groups so
        # the first projection matmuls wait on ~1MB, not the full tensors
        wq_sb = const.tile([128, CT, QH * D], BF)
        wk_sb = const.tile([128, CT, D], BF)
        wv_sb = const.tile([128, CT, D], BF)
        xTr = xT_d.rearrange("(ci p) t -> p ci t", p=128)
        wqr = wq_d.rearrange("(ci p) n -> p ci n", p=128)
        wkr = wk_d.rearrange("(ci p) n -> p ci n", p=128)
        wvr = wv_d.rearrange("(ci p) n -> p ci n", p=128)
        GW = 8
        # groups 0-1 up-front on the gpsimd queue; later groups are staged
        # into the sync queue between chunk-0 xt loads so the early xt stream
        # keeps HBM bandwidth (deferred_dma fires inside the ci loop below)
        for g in range(0, 2 * GW, GW):
            s = slice(g, g + GW)
            nc.gpsimd.dma_start(wq_sb[:, s, :], wqr[:, s, :])
            nc.gpsimd.dma_start(wk_sb[:, s, :], wkr[:, s, :])
            nc.gpsimd.dma_start(wv_sb[:, s, :], wvr[:, s, :])
        cos_sb = const.tile([D, T], BF)
        nc.gpsimd.dma_start(cos_sb[:], cos_d)
        sin_sb = const.tile([D, T], BF)
        nc.gpsimd.dma_start(sin_sb[:], sin_d)
        alw_sb = const.tile([128, 2, 1024], BF)
        nc.gpsimd.dma_start(alw_sb[:], alw_d)
        id_sb = const.tile([128, 128], BF)
        nc.gpsimd.dma_start(id_sb[:], id_d)
        wo_sb = const.tile([128, QH, C], BF)

        def stage_weights(g):
            s = slice(g, g + GW)
            nc.sync.dma_start(wq_sb[:, s, :], wqr[:, s, :])
            nc.sync.dma_start(wk_sb[:, s, :], wkr[:, s, :])
            nc.sync.dma_start(wv_sb[:, s, :], wvr[:, s, :])

        deferred_dma = {
            (0, 0, 2): lambda: stage_weights(16),
            (0, 0, 10): lambda: stage_weights(24),
            (0, 1, 8): lambda: nc.sync.dma_start(wo_sb[:], wo_d),
        }
        onesbf_sb = const.tile([128, 128], BF)
        nc.gpsimd.memset(onesbf_sb[:], 1.0)
        bias_sb = const.tile([128, 1], F32)
        nc.gpsimd.memset(bias_sb[:], EXP_BIAS)

        def rope_evict(dst, psum, cs):
            # dst = psum * cos + swap_halves(psum) * sin_rot   (bf16 out)
            # half-swap copies on ACT (fast PSUM reads), muls/add on DVE
            sw = work.tile([128, 512], F32, tag="sw", bufs=3, name="sw")
            nc.scalar.copy(sw[0:64, :], psum[64:128, :])
            nc.scalar.copy(sw[64:128, :], psum[0:64, :])
            nc.vector.tensor_mul(sw[:], sw[:], sin_sb[:, cs])
            cst = work.tile([128, 512], F32, tag="cst", bufs=3, name="cst")
            nc.vector.tensor_mul(cst[:], psum[:], cos_sb[:, cs])
            nc.vector.tensor_add(dst, cst[:], sw[:])

        wo_jobs = deque()
        # wo-job PSUM accumulators ping-pong over half-bank slots; during the
        # end-of-batch drain the idle score banks join the rotation so PE
        # never waits for an eviction
        ops_holder = {"tags": ["bk23"], "i": 0, "tiles": {}}

        def make_wo_job(b, j, tl, o, yts):
            def job():
                tags = ops_holder["tags"]
                slot = ops_holder["i"] % (2 * len(tags))
                tag = tags[slot // 2]
                if slot % 2 == 0:
                    ops_holder["tiles"][tag] = bank2(tag)
                ops = (ops_holder["tiles"][tag][:, 0:512] if slot % 2 == 0
                       else ops_holder["tiles"][tag][:, 512:1024])
                ops_holder["i"] += 1
                for h in range(QH):
                    nc.tensor.matmul(
                        ops, yts[h][:, 128 * tl:128 * (tl + 1)],
                        wo_sb[:, h, 512 * o:512 * (o + 1)],
                        start=h == 0, stop=h == QH - 1)
                ob = work.tile([128, 512], BF, tag="ob", bufs=6,
                               name="ob")
                if (tl + o) % 2 == 0:
                    nc.vector.tensor_copy(ob[:], ops)
                else:
                    nc.scalar.copy(ob[:], ops)
                r0 = b * T + 512 * j + 128 * tl
                nc.sync.dma_start(out_d[r0:r0 + 128, 512 * o:512 * (o + 1)],
                                  ob[:])
            return job

        for b in range(B):
            qT = act.tile([D, QH, T], BF, tag="qT", name="qT")
            kT = act.tile([D, T], BF, tag="kT", name="kT")
            vT = act.tile([D, T], BF, tag="vT", name="vT")
            vsb = act.tile([128, T // 128, D], BF, tag="v", name="vsb")

            # ---- projections ----
            # banks 0-1 = pq0,pq1; banks 2-3 = pq2,pq3; banks 4-5 = pk,pv
            for jc in range(NCH):
                bkA = bank2("bk01")
                bkB = bank2("bk23")
                bkC = bank2("bk45")
                pq = [bkA[:, 0:512], bkA[:, 512:1024],
                      bkB[:, 0:512], bkB[:, 512:1024]]
                pk = bkC[:, 0:512]
                pv = bkC[:, 512:1024]
                # q matmuls run SKEW c-tiles behind k/v so the previous
                # chunk's pq bank evictions are hidden behind ready work
                SKEW = 4
                xts = {}
                col0 = b * T + 512 * jc

                def q_mms(cq):
                    for h in range(QH):
                        nc.tensor.matmul(
                            pq[h], wq_sb[:, cq, 128 * h:128 * (h + 1)],
                            xts[cq], start=cq == 0, stop=cq == CT - 1)
                    if cq >= SKEW:
                        del xts[cq - SKEW]

                for ci in range(CT):
                    if (b, jc, ci) in deferred_dma:
                        deferred_dma.pop((b, jc, ci))()
                    if ci % 2 == 0:
                        # one DMA covers two contraction tiles (fewer, larger
                        # transfers keep the activation stream ahead of PE)
                        xt2 = work.tile([128, 2, 512], BF, tag="xt", bufs=6,
                                        name="xt2")
                        nc.sync.dma_start(
                            xt2[:], xTr[:, ci:ci + 2, col0:col0 + 512])
                        xts[ci] = xt2[:, 0, :]
                        xts[ci + 1] = xt2[:, 1, :]
                    st, sp = ci == 0, ci == CT - 1
                    nc.tensor.matmul(pk, wk_sb[:, ci, :], xts[ci],
                                     start=st, stop=sp)
                    nc.tensor.matmul(pv, wv_sb[:, ci, :], xts[ci],
                                     start=st, stop=sp)
                    if ci >= SKEW:
                        q_mms(ci - SKEW)
                for cq in range(CT - SKEW, CT):
                    q_mms(cq)
                cs = slice(512 * jc, 512 * (jc + 1))
                nc.scalar.copy(vT[:, cs], pv)
                last = jc == NCH - 1
                if last:
                    # transposes first (banks 6-7, free now) so attention can
                    # start while the last chunk's RoPE evictions trail
                    bkDv = bank2("bk67").bitcast(BF)
                    for k in range(T // 128):
                        tp = (bkDv[:, 0:128] if k % 2 == 0
                              else bkDv[:, 1024:1152])
                        nc.tensor.transpose(tp, vT[:, 128 * k:128 * (k + 1)],
                                            id_sb[:])
                        if k % 2 == 0:
                            nc.vector.tensor_copy(vsb[:, k, :], tp)
                        else:
                            nc.scalar.copy(vsb[:, k, :], tp)
                    rope_evict(qT[:, 0, cs], pq[0], cs)
                    rope_evict(qT[:, 1, cs], pq[1], cs)
                    rope_evict(kT[:, cs], pk, cs)
                    rope_evict(qT[:, 2, cs], pq[2], cs)
                    rope_evict(qT[:, 3, cs], pq[3], cs)
                else:
                    rope_evict(kT[:, cs], pk, cs)
                    for h in range(QH):
                        rope_evict(qT[:, h, cs], pq[h], cs)

            # ---- attention + output projection ----
            # banks 0-1 = yps,dps; banks 2-3 = wo accumulators; 4-7 = scores
            for j in range(NCH):
                yts = {}
                K = 4 * j + 4
                P = K // 2
                for h in range(QH):
                    # denominator pre-sum engine: DVE for the first/last head
                    # (short latency to the ones-matmul), GPSIMD for the rest
                    eng = nc.vector if h in (0, QH - 1) else nc.gpsimd
                    qs = qT[:, h, 512 * j:512 * (j + 1)]
                    # pass 1: paired score matmuls stream; paired exp trails
                    # on ACT; pair-level key pre-sum trails on DVE/GPSIMD
                    pts = []
                    accp = None
                    npop = 2 if j in (0, NCH - 1) else 1
                    for p in range(P):
                        sps = bank2("bk67") if p % 2 == 0 else bank2("bk45")
                        # skip score columns that the causal mask fully
                        # zeroes anyway (the -30000 additive mask covers the
                        # stale PSUM there); k-tile 4j+ot masks cols < 128*ot
                        for half in range(2):
                            kt = 2 * p + half
                            skip = max(0, 128 * (kt - 4 * j))
                            nc.tensor.matmul(
                                sps[:, 512 * half + skip:512 * (half + 1)],
                                kT[:, 128 * kt:128 * (kt + 1)],
                                qs[:, skip:512], start=True, stop=True)
                        # additive causal mask applied to the PSUM scores
                        # (cheap DVE PSUM op; keeps the post-exp path clean)
                        o = 2 * p - 4 * j
                        if o >= 0:
                            nc.vector.tensor_add(sps[:], sps[:],
                                                 alw_sb[:, o // 2, :])
                        # pop PE jobs next: their PSUM evictions land ahead
                        # of this pair's exp-gated ops in the engine FIFOs.
                        # bk01 (yps/dps) is idle during pass 1, so jobs may
                        # rotate over it too (deeper eviction pipeline);
                        # pop rate per chunk spreads the 32 jobs evenly
                        ops_holder["tags"] = ["bk23", "bk01"]
                        for _ in range(npop):
                            if wo_jobs:
                                wo_jobs.popleft()()
                        pt = work.tile([128, 1024], BF, tag="pt", bufs=10,
                                       name="pt")
                        nc.scalar.activation(pt[:], sps[:], AFT.Exp,
                                             bias=bias_sb[:], scale=SCALE)
                        # bf16 ping-pong pair-accumulator (out-of-place: DVE
                        # 2x packing; final add always on DVE for low latency
                        # to the denominator matmuls)
                        e = nc.vector if p == P - 1 else eng
                        if p == 1:
                            accp = work.tile([128, 1024], BF, tag="accp",
                                             bufs=8, name="accp")
                            e.tensor_add(accp[:], pts[0][:], pt[:])
                        elif p > 1:
                            accp2 = work.tile([128, 1024], BF, tag="accp",
                                              bufs=8, name="accp")
                            e.tensor_add(accp2[:], accp[:], pt[:])
                            accp = accp2
                        pts.append(pt)
                    # pass 2: attn@v accumulation (dense PE). Jobs must NOT
                    # touch bk01 from here on: yps/dps live there and their
                    # readers are emitted after the pops (deadlock otherwise)
                    ops_holder["tags"] = ["bk23"]
                    bkY = bank2("bk01")
                    yps = bkY[:, 0:512]
                    dps = bkY[:, 512:1024]
                    for k in range(K):
                        # pt is exactly zero in fully-masked columns; skip
                        # them (k == 0 is always full-width, so the start
                        # matmul initializes every column)
                        skip = max(0, 128 * (k - 4 * j))
                        nc.tensor.matmul(
                            yps[:, skip:512], vsb[:, k, :],
                            pts[k // 2][:, 512 * (k % 2) + skip:
                                        512 * (k % 2) + 512],
                            start=(k == 0), stop=(k == K - 1))
                    for _ in range(3):
                        if wo_jobs:
                            wo_jobs.popleft()()
                    # denominator: two accumulating ones-matmuls over the
                    # bf16 pair-accumulator halves (no fold needed)
                    nc.tensor.matmul(dps, onesbf_sb[:], accp[:, 0:512],
                                     start=True, stop=False)
                    nc.tensor.matmul(dps, onesbf_sb[:], accp[:, 512:1024],
                                     start=False, stop=True)
                    rec = work.tile([128, 512], F32, tag="rec", bufs=2,
                                    name="rec")
                    nc.vector.reciprocal_approx_fast(rec[:], dps)
                    yt = work.tile([128, 512], BF, tag="yt", bufs=8,
                                   name="yt")
                    nc.vector.tensor_mul(yt[:], yps, rec[:])
                    yts[h] = yt
                for tl in range(4):
                    for o in range(C // 512):
                        wo_jobs.append(make_wo_job(b, j, tl, o, yts))
            # keep a few jobs alive across the batch boundary so the next
            # batch's first attention chunk has dense PE filler work
            keep = 12 if b < B - 1 else 0
            ops_holder["tags"] = ["bk23", "bk45", "bk67", "bk01"]
            ops_holder["i"] = 0
            while len(wo_jobs) > keep:
                wo_jobs.popleft()()
            ops_holder["tags"] = ["bk23"]
            ops_holder["i"] = 0

    nc.compile()
    return nc


def host_prep(inputs):
    x = np.asarray(inputs["x"], np.float32)
    mask = np.asarray(inputs["mask"], np.float32)
    wq = np.asarray(inputs["wq"], np.float32)
    wk = np.asarray(inputs["wk"], np.float32)
    wv = np.asarray(inputs["wv"], np.float32)
    wo = np.asarray(inputs["wo"], np.float32)

    xT = np.ascontiguousarray(x.reshape(B * T, C).T).astype(bf16)
    inv = 1.0 / (ROPE_BASE ** (np.arange(0, D, 2, dtype=np.float64) / D))
    freqs = np.arange(T, dtype=np.float64)[:, None] * inv[None, :] * B
    emb = np.concatenate([freqs, freqs], axis=-1)       # [T, D]
    cosT = np.cos(emb).T.astype(np.float32).astype(bf16)
    sinT = np.sin(emb).T.astype(np.float32)
    sinT[: D // 2] *= -1.0
    sinTr = sinT.astype(bf16)
    # additive causal mask: -30000 where mask[jj, 128*o + p] says "disallow",
    # 0 elsewhere; stored as two k-tile PAIRS so one DVE add masks a whole
    # [128,1024] PSUM score pair before the exp
    allowA = np.stack([(1.0 - mask[0:512, 128 * o:128 * (o + 1)]).T
                       for o in range(4)], axis=1)            # [128, 4, 512]
    negP = np.ascontiguousarray(
        (allowA.reshape(128, 2, 1024) - 1.0) * 30000.0).astype(bf16)
    ident = np.eye(128, dtype=np.float32).astype(bf16)

    common = dict(xT=xT, cosT=cosT, sinTr=sinTr, negP=negP, ident=ident)
    in_maps = []
    for c in range(NCORES):
        m = dict(common)
        m["wq"] = np.ascontiguousarray(wq[:, 512 * c:512 * (c + 1)]).astype(bf16)
        m["wk"] = np.ascontiguousarray(wk[:, 128 * c:128 * (c + 1)]).astype(bf16)
        m["wv"] = np.ascontiguousarray(wv[:, 128 * c:128 * (c + 1)]).astype(bf16)
        m["woA"] = np.ascontiguousarray(
            wo[512 * c:512 * (c + 1), :].reshape(QH, 128, C)
            .transpose(1, 0, 2)).astype(bf16)
        in_maps.append(m)
    return in_maps


def kernel(**inputs) -> np.ndarray:
    from concourse.bass_utils import run_bass_kernel_spmd

    in_maps = host_prep(inputs)
    nc = emit_program()
    trace = bool(os.environ.get("BASS_KERNEL_TRACE"))
    res = run_bass_kernel_spmd(nc, in_maps, core_ids=list(range(NCORES)),
                               trace=trace)
    if trace and res.exec_time_ns is not None:
        print(f"HW exec time: {res.exec_time_ns} ns")
        if res.instructions_and_trace is not None:
            print("trace:", res.instructions_and_trace[1])
    total = np.zeros((B * T, C), np.float32)
    for r in res.results:
        total += np.asarray(r["out"], np.float32)
    return total.reshape(B, T, C)


# revision 34
# speedup vs baseline: 1.1609x; 1.0162x over previous
"""Trainium2 Bass kernel for GQA attention (B=2, T=2048, C=4096, H=32, KV=8, D=128)
with RoPE and causal mask.

Sharding: tensor-parallel over heads across 8 cores. Each core owns 4 Q heads and
their shared KV head: projects q/k/v for those heads, runs causal attention, and
computes a partial output projection; the host sums the 8 partials (bf16).

All on-chip layouts are transposed ([feature, token]) so every matmul consumes
natural slices:
  qT/kT/vT = W^T @ x  via lhsT=W-tile [128c, cols], rhs=xT-tile [128c, 512t]
  sT[tk, tq] = kT-tile^T @ qT-chunk, two k-tiles paired into one [128,1024]
  PSUM tile so ACT computes exp on both halves in one ACTIVATE
  pT = exp(sT/sqrt(D) - 10); strictly-causal-upper tiles skipped entirely
  yT[d, tq] += v-tile^T @ pT          (v pre-transposed to [t, d] via PE transpose)
  out[tq, :] += yT_h^T @ wo_h         (accumulate 4 heads in PSUM, evict bf16)

Softmax denominator: the exp tiles are pre-summed across key-tiles on DVE
(heads 0,2) / GPSIMD (heads 1,3) into a bf16 row-sum; a single ones-matmul per
(head, chunk) reduces it across partitions. This removes the per-key-tile
ones-matmul stream (~9% of PE cycles) from the tensor engine.

PSUM is managed as ONE kernel-wide pool of four 2-bank [128,1024] tags so no
pool-release barrier ever serializes phase transitions (per-address WAR deps
only): banks 0-1 = pq0/pq1 then yps/dps; banks 2-3 = pq2/pq3 then the wo-job
accumulators; banks 4-5 = pk/pv then odd score pairs; banks 6-7 = V-transpose
ping-pong then even score pairs. At the end of each batch's projections the
V transposes + their evictions are emitted BEFORE the last chunk's RoPE
evictions, so the first attention scores (banks 6-7) start within ~4us of the
last projection matmul and the PE never goes HAM-cold at the phase boundary.
Output-projection matmul "jobs" are popped from a queue inside the score
streams to keep the in-order PE queue dense while ACT works through the exps.
"""

import os
from collections import deque
from contextlib import ExitStack

import numpy as np
import ml_dtypes

import concourse.bacc as bacc
import concourse.mybir as mybir
import concourse.tile as tile

BF = mybir.dt.bfloat16
F32 = mybir.dt.float32
AFT = mybir.ActivationFunctionType

NCORES = 8
B, T, C = 2, 2048, 4096
H, KV, D = 32, 8, 128
QH = H // NCORES          # 4 q-heads per core
CT = C // 128             # 32 contraction tiles
NCH = T // 512            # 4 query chunks per batch
SCALE = 1.0 / float(np.sqrt(D))
EXP_BIAS = -10.0
ROPE_BASE = 10000.0

bf16 = ml_dtypes.bfloat16


def emit_program():
    nc = bacc.Bacc("TRN2", target_bir_lowering=False, debug=False,
                   num_devices=NCORES)

    xT_d = nc.dram_tensor("xT", [C, B * T], BF, kind="ExternalInput").ap()
    wq_d = nc.dram_tensor("wq", [C, QH * D], BF, kind="ExternalInput").ap()
    wk_d = nc.dram_tensor("wk", [C, D], BF, kind="ExternalInput").ap()
    wv_d = nc.dram_tensor("wv", [C, D], BF, kind="ExternalInput").ap()
    wo_d = nc.dram_tensor("woA", [128, QH, C], BF, kind="ExternalInput").ap()
    cos_d = nc.dram_tensor("cosT", [D, T], BF, kind="ExternalInput").ap()
    sin_d = nc.dram_tensor("sinTr", [D, T], BF, kind="ExternalInput").ap()
    alw_d = nc.dram_tensor("negP", [128, 2, 1024], BF, kind="ExternalInput").ap()
    id_d = nc.dram_tensor("ident", [128, 128], BF, kind="ExternalInput").ap()
    out_d = nc.dram_tensor("out", [B * T, C], BF, kind="ExternalOutput").ap()

    with tile.TileContext(nc) as tc, ExitStack() as ctx:
        const = ctx.enter_context(tc.tile_pool(name="const", bufs=1))
        act = ctx.enter_context(tc.tile_pool(name="act", bufs=1))
        work = ctx.enter_context(tc.tile_pool(name="work", bufs=1))
        ps = ctx.enter_context(tc.tile_pool(name="ps", bufs=1, space="PSUM"))

        def bank2(tag):
            return ps.tile([128, 1024], F32, tag=tag, bufs=1, name=tag)

        # weights + tables on the gpsimd DMA queue so they never sit ahead of
        # the xt activation loads (sync queue); chunked in 8-c-tile groups so
        # the first projection matmuls wait on ~1MB, not the full tensors
        wq_sb = const.tile([128, CT, QH * D], BF)
        wk_sb = const.tile([128, CT, D], BF)
        wv_sb = const.tile([128, CT, D], BF)
        xTr = xT_d.rearrange("(ci p) t -> p ci t", p=128)
        wqr = wq_d.rearrange("(ci p) n -> p ci n", p=128)
        wkr = wk_d.rearrange("(ci p) n -> p ci n", p=128)
        wvr = wv_d.rearrange("(ci p) n -> p ci n", p=128)
        GW = 8
        # groups 0-1 up-front on the gpsimd queue; later groups are staged
        # into the sync queue between chunk-0 xt loads so the early xt stream
        # keeps HBM bandwidth (deferred_dma fires inside the ci loop below)
        for g in range(0, 2 * GW, GW):
            s = slice(g, g + GW)
            nc.gpsimd.dma_start(wq_sb[:, s, :], wqr[:, s, :])
            nc.gpsimd.dma_start(wk_sb[:, s, :], wkr[:, s, :])
            nc.gpsimd.dma_start(wv_sb[:, s, :], wvr[:, s, :])
        cos_sb = const.tile([D, T], BF)
        nc.gpsimd.dma_start(cos_sb[:], cos_d)
        sin_sb = const.tile([D, T], BF)
        nc.gpsimd.dma_start(sin_sb[:], sin_d)
        alw_sb = const.tile([128, 2, 1024], BF)
        nc.gpsimd.dma_start(alw_sb[:], alw_d)
        id_sb = const.tile([128, 128], BF)
        nc.gpsimd.dma_start(id_sb[:], id_d)
        wo_sb = const.tile([128, QH, C], BF)

        def stage_weights(g):
            s = slice(g, g + GW)
            nc.sync.dma_start(wq_sb[:, s, :], wqr[:, s, :])
            nc.sync.dma_start(wk_sb[:, s, :], wkr[:, s, :])
            nc.sync.dma_start(wv_sb[:, s, :], wvr[:, s, :])

        deferred_dma = {
            (0, 0, 2): lambda: stage_weights(16),
            (0, 0, 10): lambda: stage_weights(24),
            (0, 1, 8): lambda: nc.sync.dma_start(wo_sb[:], wo_d),
        }
        onesbf_sb = const.tile([128, 128], BF)
        nc.gpsimd.memset(onesbf_sb[:], 1.0)
        bias_sb = const.tile([128, 1], F32)
        nc.gpsimd.memset(bias_sb[:], EXP_BIAS)

        def rope_evict(dst, psum, cs):
            # dst = psum * cos + swap_halves(psum) * sin_rot   (bf16 out)
            # half-swap copies on ACT (fast PSUM reads), muls/add on DVE
            sw = work.tile([128, 512], F32, tag="sw", bufs=3, name="sw")
            nc.scalar.copy(sw[0:64, :], psum[64:128, :])
            nc.scalar.copy(sw[64:128, :], psum[0:64, :])
            nc.vector.tensor_mul(sw[:], sw[:], sin_sb[:, cs])
            cst = work.tile([128, 512], F32, tag="cst", bufs=3, name="cst")
            nc.vector.tensor_mul(cst[:], psum[:], cos_sb[:, cs])
            nc.vector.tensor_add(dst, cst[:], sw[:])

        wo_jobs = deque()
        # wo-job PSUM accumulators ping-pong over half-bank slots; during the
        # end-of-batch drain the idle score banks join the rotation so PE
        # never waits for an eviction
        ops_holder = {"tags": ["bk23"], "i": 0, "tiles": {}}

        def make_wo_job(b, j, tl, o, yts):
            def job():
                tags = ops_holder["tags"]
                slot = ops_holder["i"] % (2 * len(tags))
                tag = tags[slot // 2]
                if slot % 2 == 0:
                    ops_holder["tiles"][tag] = bank2(tag)
                ops = (ops_holder["tiles"][tag][:, 0:512] if slot % 2 == 0
                       else ops_holder["tiles"][tag][:, 512:1024])
                ops_holder["i"] += 1
                for h in range(QH):
                    nc.tensor.matmul(
                        ops, yts[h][:, 128 * tl:128 * (tl + 1)],
                        wo_sb[:, h, 512 * o:512 * (o + 1)],
                        start=h == 0, stop=h == QH - 1)
                ob = work.tile([128, 512], BF, tag="ob", bufs=6,
                               name="ob")
                if (tl + o) % 2 == 0:
                    nc.vector.tensor_copy(ob[:], ops)
                else:
                    nc.scalar.copy(ob[:], ops)
                r0 = b * T + 512 * j + 128 * tl
                nc.sync.dma_start(out_d[r0:r0 + 128, 512 * o:512 * (o + 1)],
                                  ob[:])
            return job

        for b in range(B):
            qT = act.tile([D, QH, T], BF, tag="qT", name="qT")
            kT = act.tile([D, T], BF, tag="kT", name="kT")
            vT = act.tile([D, T], BF, tag="vT", name="vT")
            vsb = act.tile([128, T // 128, D], BF, tag="v", name="vsb")

            # ---- projections ----
            # banks 0-1 = pq0,pq1; banks 2-3 = pq2,pq3; banks 4-5 = pk,pv
            for jc in range(NCH):
                bkA = bank2("bk01")
                bkB = bank2("bk23")
                bkC = bank2("bk45")
                pq = [bkA[:, 0:512], bkA[:, 512:1024],
                      bkB[:, 0:512], bkB[:, 512:1024]]
                pk = bkC[:, 0:512]
                pv = bkC[:, 512:1024]
                # q matmuls run SKEW c-tiles behind k/v so the previous
                # chunk's pq bank evictions are hidden behind ready work
                SKEW = 4
                xts = {}
                col0 = b * T + 512 * jc

                def q_mms(cq):
                    for h in range(QH):
                        nc.tensor.matmul(
                            pq[h], wq_sb[:, cq, 128 * h:128 * (h + 1)],
                            xts[cq], start=cq == 0, stop=cq == CT - 1)
                    if cq >= SKEW:
                        del xts[cq - SKEW]

                for ci in range(CT):
                    if (b, jc, ci) in deferred_dma:
                        deferred_dma.pop((b, jc, ci))()
                    if ci % 2 == 0:
                        # one DMA covers two contraction tiles (fewer, larger
                        # transfers keep the activation stream ahead of PE)
                        xt2 = work.tile([128, 2, 512], BF, tag="xt", bufs=6,
                                        name="xt2")
                        nc.sync.dma_start(
                            xt2[:], xTr[:, ci:ci + 2, col0:col0 + 512])
                        xts[ci] = xt2[:, 0, :]
                        xts[ci + 1] = xt2[:, 1, :]
                    st, sp = ci == 0, ci == CT - 1
                    nc.tensor.matmul(pk, wk_sb[:, ci, :], xts[ci],
                                     start=st, stop=sp)
                    nc.tensor.matmul(pv, wv_sb[:, ci, :], xts[ci],
                                     start=st, stop=sp)
                    if ci >= SKEW:
                        q_mms(ci - SKEW)
                for cq in range(CT - SKEW, CT):
                    q_mms(cq)
                cs = slice(512 * jc, 512 * (jc + 1))
                nc.scalar.copy(vT[:, cs], pv)
                last = jc == NCH - 1
                if last:
                    # transposes first (banks 6-7, free now) so attention can
                    # start while the last chunk's RoPE evictions trail
                    bkDv = bank2("bk67").bitcast(BF)
                    for k in range(T // 128):
                        tp = (bkDv[:, 0:128] if k % 2 == 0
                              else bkDv[:, 1024:1152])
                        nc.tensor.transpose(tp, vT[:, 128 * k:128 * (k + 1)],
                                            id_sb[:])
                        if k % 2 == 0:
                            nc.vector.tensor_copy(vsb[:, k, :], tp)
                        else:
                            nc.scalar.copy(vsb[:, k, :], tp)
                    rope_evict(qT[:, 0, cs], pq[0], cs)
                    rope_evict(qT[:, 1, cs], pq[1], cs)
                    rope_evict(kT[:, cs], pk, cs)
                    rope_evict(qT[:, 2, cs], pq[2], cs)
                    rope_evict(qT[:, 3, cs], pq[3], cs)
                else:
                    rope_evict(kT[:, cs], pk, cs)
                    for h in range(QH):
                        rope_evict(qT[:, h, cs], pq[h], cs)

            # ---- attention + output projection ----
            # banks 0-1 = yps,dps; banks 2-3 = wo accumulators; 4-7 = scores
            for j in range(NCH):
                yts = {}
                K = 4 * j + 4
                P = K // 2
                for h in range(QH):
                    # denominator pre-sum engine: DVE for the first/last head
                    # (short latency to the ones-matmul), GPSIMD for the rest
                    eng = nc.vector if h in (0, QH - 1) else nc.gpsimd
                    qs = qT[:, h, 512 * j:512 * (j + 1)]
                    # pass 1: paired score matmuls stream; paired exp trails
                    # on ACT; pair-level key pre-sum trails on DVE/GPSIMD
                    pts = []
                    accp = None
                    npop = 2 if j == 0 else 1
                    for p in range(P):
                        sps = bank2("bk67") if p % 2 == 0 else bank2("bk45")
                        # skip score columns that the causal mask fully
                        # zeroes anyway (the -30000 additive mask covers the
                        # stale PSUM there); k-tile 4j+ot masks cols < 128*ot
                        for half in range(2):
                            kt = 2 * p + half
                            skip = max(0, 128 * (kt - 4 * j))
                            nc.tensor.matmul(
                                sps[:, 512 * half + skip:512 * (half + 1)],
                                kT[:, 128 * kt:128 * (kt + 1)],
                                qs[:, skip:512], start=True, stop=True)
                        # additive causal mask applied to the PSUM scores
                        # (cheap DVE PSUM op; keeps the post-exp path clean)
                        o = 2 * p - 4 * j
                        if o >= 0:
                            nc.vector.tensor_add(sps[:], sps[:],
                                                 alw_sb[:, o // 2, :])
                        # pop PE jobs next: their PSUM evictions land ahead
                        # of this pair's exp-gated ops in the engine FIFOs.
                        # bk01 (yps/dps) is idle during pass 1, so jobs may
                        # rotate over it too (deeper eviction pipeline);
                        # pop rate per chunk spreads the 32 jobs evenly
                        ops_holder["tags"] = ["bk23", "bk01"]
                        for _ in range(npop):
                            if wo_jobs:
                                wo_jobs.popleft()()
                        pt = work.tile([128, 1024], BF, tag="pt", bufs=10,
                                       name="pt")
                        if j == 0:
                            # boundary: hoist the first exps ahead of the
                            # previous chunk's trailing RoPE copies on ACT
                            with tc.high_priority(offset=80):
                                nc.scalar.activation(pt[:], sps[:], AFT.Exp,
                                                     bias=bias_sb[:],
                                                     scale=SCALE)
                        else:
                            nc.scalar.activation(pt[:], sps[:], AFT.Exp,
                                                 bias=bias_sb[:], scale=SCALE)
                        # bf16 ping-pong pair-accumulator (out-of-place: DVE
                        # 2x packing; final add always on DVE for low latency
                        # to the denominator matmuls)
                        e = nc.vector if p == P - 1 else eng
                        if p == 1:
                            accp = work.tile([128, 1024], BF, tag="accp",
                                             bufs=8, name="accp")
                            e.tensor_add(accp[:], pts[0][:], pt[:])
                        elif p > 1:
                            accp2 = work.tile([128, 1024], BF, tag="accp",
                                              bufs=8, name="accp")
                            e.tensor_add(accp2[:], accp[:], pt[:])
                            accp = accp2
                        pts.append(pt)
                    # pass 2: attn@v accumulation (dense PE). Jobs must NOT
                    # touch bk01 from here on: yps/dps live there and their
                    # readers are emitted after the pops (deadlock otherwise)
                    ops_holder["tags"] = ["bk23"]
                    bkY = bank2("bk01")
                    yps = bkY[:, 0:512]
                    dps = bkY[:, 512:1024]
                    for k in range(K):
                        # pt is exactly zero in fully-masked columns; skip
                        # them (k == 0 is always full-width, so the start
                        # matmul initializes every column)
                        skip = max(0, 128 * (k - 4 * j))
                        nc.tensor.matmul(
                            yps[:, skip:512], vsb[:, k, :],
                            pts[k // 2][:, 512 * (k % 2) + skip:
                                        512 * (k % 2) + 512],
                            start=(k == 0), stop=(k == K - 1))
                    for _ in range(4 if j == NCH - 1 else 3):
                        if wo_jobs:
                            wo_jobs.popleft()()
                    # denominator: two accumulating ones-matmuls over the
                    # bf16 pair-accumulator halves (no fold needed)
                    nc.tensor.matmul(dps, onesbf_sb[:], accp[:, 0:512],
                                     start=True, stop=False)
                    nc.tensor.matmul(dps, onesbf_sb[:], accp[:, 512:1024],
                                     start=False, stop=True)
                    rec = work.tile([128, 512], F32, tag="rec", bufs=2,
                                    name="rec")
                    nc.vector.reciprocal_approx_fast(rec[:], dps)
                    yt = work.tile([128, 512], BF, tag="yt", bufs=8,
                                   name="yt")
                    nc.vector.tensor_mul(yt[:], yps, rec[:])
                    yts[h] = yt
                for tl in range(4):
                    for o in range(C // 512):
                        wo_jobs.append(make_wo_job(b, j, tl, o, yts))
            # keep a few jobs alive across the batch boundary so the next
            # batch's first attention chunk has dense PE filler work
            keep = 12 if b < B - 1 else 0
            ops_holder["tags"] = ["bk23", "bk45", "bk67", "bk01"]
            ops_holder["i"] = 0
            while len(wo_jobs) > keep:
                wo_jobs.popleft()()
            ops_holder["tags"] = ["bk23"]
            ops_holder["i"] = 0

    nc.compile()
    return nc


def host_prep(inputs):
    x = np.asarray(inputs["x"], np.float32)
    mask = np.asarray(inputs["mask"], np.float32)
    wq = np.asarray(inputs["wq"], np.float32)
    wk = np.asarray(inputs["wk"], np.float32)
    wv = np.asarray(inputs["wv"], np.float32)
    wo = np.asarray(inputs["wo"], np.float32)

    xT = np.ascontiguousarray(x.reshape(B * T, C).T).astype(bf16)
    inv = 1.0 / (ROPE_BASE ** (np.arange(0, D, 2, dtype=np.float64) / D))
    freqs = np.arange(T, dtype=np.float64)[:, None] * inv[None, :] * B
    emb = np.concatenate([freqs, freqs], axis=-1)       # [T, D]
    cosT = np.cos(emb).T.astype(np.float32).astype(bf16)
    sinT = np.sin(emb).T.astype(np.float32)
    sinT[: D // 2] *= -1.0
    sinTr = sinT.astype(bf16)
    # additive causal mask: -30000 where mask[jj, 128*o + p] says "disallow",
    # 0 elsewhere; stored as two k-tile PAIRS so one DVE add masks a whole
    # [128,1024] PSUM score pair before the exp
    allowA = np.stack([(1.0 - mask[0:512, 128 * o:128 * (o + 1)]).T
                       for o in range(4)], axis=1)            # [128, 4, 512]
    negP = np.ascontiguousarray(
        (allowA.reshape(128, 2, 1024) - 1.0) * 30000.0).astype(bf16)
    ident = np.eye(128, dtype=np.float32).astype(bf16)

    common = dict(xT=xT, cosT=cosT, sinTr=sinTr, negP=negP, ident=ident)
    in_maps = []
    for c in range(NCORES):
        m = dict(common)
        m["wq"] = np.ascontiguousarray(wq[:, 512 * c:512 * (c + 1)]).astype(bf16)
        m["wk"] = np.ascontiguousarray(wk[:, 128 * c:128 * (c + 1)]).astype(bf16)
        m["wv"] = np.ascontiguousarray(wv[:, 128 * c:128 * (c + 1)]).astype(bf16)
        m["woA"] = np.ascontiguousarray(
            wo[512 * c:512 * (c + 1), :].reshape(QH, 128, C)
            .transpose(1, 0, 2)).astype(bf16)
        in_maps.append(m)
    return in_maps


def kernel(**inputs) -> np.ndarray:
    from concourse.bass_utils import run_bass_kernel_spmd

    in_maps = host_prep(inputs)
    nc = emit_program()
    trace = bool(os.environ.get("BASS_KERNEL_TRACE"))
    res = run_bass_kernel_spmd(nc, in_maps, core_ids=list(range(NCORES)),
                               trace=trace)
    if trace and res.exec_time_ns is not None:
        print(f"HW exec time: {res.exec_time_ns} ns")
        if res.instructions_and_trace is not None:
            print("trace:", res.instructions_and_trace[1])
    total = np.zeros((B * T, C), np.float32)
    for r in res.results:
        total += np.asarray(r["out"], np.float32)
    return total.reshape(B, T, C)
